# revision 12
# baseline (speedup 1.0000x reference)
"""Trainium2 Bass kernel for causal self-attention (B=2, S=2048, D=1024, H=16).

Sharding: 8 cores = 2 batch groups x 4 head-groups (tensor parallel).
Core c handles batch b = c // 4 and heads [4*(c%4), 4*(c%4)+4).
Each core computes a partial out-projection [S, D]; the host sums the 4
partials of each batch group (row-parallel TP unshard) and adds bout.

Per-core pipeline (all layouts chosen so no on-device transposes of
activations are needed except small V blocks):
  1. qkvT[col, s] = Wqkv_local.T @ x.T   (x passed pre-transposed, a host
     layout choice; weights are naturally [D, cols] = lhsT layout)
  2. scoresT[k, q] = K_h^T.T @ Q_h per 128-wide k-chunk, causal blocks only.
     Key-padding mask + 1/sqrt(64) scale fold into the ACT exp (per-partition
     bias = per-k bias in this transposed layout).  P = exp(scores') in bf16.
  3. attT[65, q] = V_ext^T @ P  where V_ext = [V_h | ones]: row 64 is the
     softmax denominator.  No separate reduction needed.
  4. normalize per-q: recip = 1/(den + eps) replicated via a K=1 matmul;
     att_n = attT * recip; query-padding mask applied as one big multiply.
  5. out_partial[s, :] = att_n.T @ Wout_local  (att_n is already the lhsT
     layout needed), DMA PSUM -> DRAM directly.
"""

import os
import sys

import numpy as np

for _p in ("/opt/trn_rl_repo",):
    if _p not in sys.path and os.path.isdir(_p):
        sys.path.insert(0, _p)

import concourse.bass as bass
import concourse.mybir as mybir
from concourse import tile
from concourse.bass_utils import run_bass_kernel_spmd

B, S, D, H = 2, 2048, 1024, 16
HD = D // H  # 64
HEADS_PER_CORE = 4
CORES = 8
LOCAL_COLS = 3 * HEADS_PER_CORE * HD  # 768 (q|k|v for 4 heads)
NEG = -1.0e30
EPS = 1.0e-30

F32 = mybir.dt.float32
F32R = mybir.dt.float32r
BF16 = mybir.dt.bfloat16

AF = mybir.ActivationFunctionType

N_STILE = 4  # 512-wide s tiles
N_KCH = S // 128  # 16 k-chunks
VEXT_W = HEADS_PER_CORE * (HD + 1)  # 260


def round_f32r(a):
    """Round fp32 array to fp32r (11-bit mantissa, round-to-nearest-even)."""
    u = np.ascontiguousarray(a, np.float32).view(np.uint32)
    low = u & np.uint32(0x00000FFF)
    base = u & np.uint32(0xFFFFF000)
    lsb = (u >> np.uint32(12)) & np.uint32(1)
    up = (low > 0x800) | ((low == 0x800) & (lsb == 1))
    return (base + (up.astype(np.uint32) << np.uint32(12))).view(np.float32)



def _split_waits(nc, cap=1):
    """Walrus in this container allows few sync-waits per instruction.
    Hoist excess waits onto preceding same-engine NoOps (same sequencer,
    program order => semantics preserved).  fp32-path Matmult lowers to
    LDW+MM whose LW struct takes no waits at all -> cap 0."""
    uid = [0]
    for fn in nc.m.functions:
        for bb in fn.blocks:
            insts = bb.instructions
            out = []
            for ins in insts:
                icap = 0 if isinstance(ins, mybir.InstMatmult) else cap
                si = ins.sync_info
                waits = list(si.on_wait) if (si and si.on_wait) else []
                if len(waits) > icap:
                    extra = waits[:-icap] if icap else waits
                    keep = waits[-icap:] if icap else []
                    gcap = max(cap, 1)
                    for i in range(0, len(extra), gcap):
                        grp = extra[i : i + gcap]
                        nop = mybir.InstNoOp(
                            name=f"wsplit-{uid[0]}", ins=[], outs=[]
                        )
                        uid[0] += 1
                        nop.engine = ins.engine
                        nop.sync_info = mybir.SyncInfo(on_wait=grp, on_update=[])
                        out.append(nop)
                    si.on_wait = keep
                out.append(ins)
            if len(out) != len(insts):
                insts[:] = out
    return nc


def build_nc(mm_dt="f32r", p_dt="bf16", split_waits=True):
    """Build the SPMD single-core program (same program on all 8 cores)."""
    nc = bass.Bass()
    mdt = F32R if mm_dt == "f32r" else F32
    pdt = BF16 if p_dt == "bf16" else F32

    xT = nc.dram_tensor("xT", [D, S], mdt, kind="ExternalInput")
    wqkv = nc.dram_tensor("wqkv", [D, LOCAL_COLS], mdt, kind="ExternalInput")
    bqkv_pc = nc.dram_tensor("bqkv_pc", [128, 6], F32, kind="ExternalInput")
    wout = nc.dram_tensor("wout", [256, D], mdt, kind="ExternalInput")
    kbias = nc.dram_tensor("kbias", [128, N_KCH], F32, kind="ExternalInput")
    qmask_rep = nc.dram_tensor("qmask_rep", [128, S], F32, kind="ExternalInput")
    tri = nc.dram_tensor("tri", [128, 128], F32, kind="ExternalInput")
    ones64 = nc.dram_tensor("ones64", [1, 64], mdt, kind="ExternalInput")
    ident = nc.dram_tensor("ident", [128, 128], mdt, kind="ExternalInput")
    out = nc.dram_tensor("out", [S, D], F32, kind="ExternalOutput")

    with tile.TileContext(nc) as tc:
        with (
            tc.tile_pool(name="consts", bufs=1) as consts,
            tc.tile_pool(name="persist", bufs=1) as persist,
        ):
            # ---- constants / persistent SBUF ----
            wqkv_sb = consts.tile([128, 8 * LOCAL_COLS], mdt)  # 8 d-chunks
            for d in range(8):
                nc.sync.dma_start(
                    wqkv_sb[:, d * LOCAL_COLS : (d + 1) * LOCAL_COLS],
                    wqkv[d * 128 : (d + 1) * 128, :],
                )
            bqkv_sb = consts.tile([128, 6], F32)
            nc.sync.dma_start(bqkv_sb[:], bqkv_pc[:])
            wout_sb = consts.tile([128, 2 * D], mdt)
            for ch in range(2):
                nc.sync.dma_start(
                    wout_sb[:, ch * D : (ch + 1) * D],
                    wout[ch * 128 : (ch + 1) * 128, :],
                )
            kbias_sb = consts.tile([128, N_KCH], F32)
            nc.sync.dma_start(kbias_sb[:], kbias[:])
            qmask_sb = consts.tile([128, S], F32)
            nc.sync.dma_start(qmask_sb[:], qmask_rep[:])
            tri_sb = consts.tile([128, 128], F32)
            nc.sync.dma_start(tri_sb[:], tri[:])
            ones_sb = consts.tile([1, 64], mdt)
            nc.sync.dma_start(ones_sb[:], ones64[:])
            ident_sb = consts.tile([128, 128], mdt)
            nc.sync.dma_start(ident_sb[:], ident[:])

            # qkvT: 6 col-chunks x [128, S]; cols c*128+p of local qkv.
            # chunks 0,1 = q (heads 01|23), 2,3 = k, 4,5 = v.
            qkvT = persist.tile([128, 6 * S], mdt)
            # V_ext: per k-chunk [128, 260]: 4 heads x (64 V cols + ones col)
            v_ext = persist.tile([128, N_KCH * VEXT_W], pdt)
            # att_n: normalized attended, transposed: 2 chunks [128, S]
            att_n = persist.tile([128, 2 * S], mdt)

            # ==================== Phase A: QKV ====================
            with (
                tc.tile_pool(name="xs", bufs=3) as xs,
                tc.tile_pool(name="qkv_ps", bufs=6, space="PSUM") as qkv_ps,
                tc.tile_pool(name="tr_ps", bufs=2, space="PSUM") as tr_ps,
            ):
                for t in range(N_STILE):
                    ps = [qkv_ps.tile([128, 512], F32, tag="qkvps", name=f"qkvps_{t}_{i}") for i in range(6)]
                    for d in range(8):
                        xt = xs.tile([128, 512], mdt, tag="xs", name=f"xs_{t}_{d}")
                        nc.gpsimd.dma_start(
                            xt[:], xT[d * 128 : (d + 1) * 128, t * 512 : (t + 1) * 512]
                        )
                        for cc in range(6):
                            nc.tensor.matmul(
                                ps[cc][:],
                                (wqkv_sb[:, d * LOCAL_COLS + cc * 128 : d * LOCAL_COLS + (cc + 1) * 128]),
                                (xt[:]),
                                start=(d == 0),
                                stop=(d == 7),
                            )
                    for cc in range(6):
                        nc.scalar.activation(
                            qkvT[:, cc * S + t * 512 : cc * S + (t + 1) * 512],
                            ps[cc][:],
                            AF.Identity,
                            bias=bqkv_sb[:, cc : cc + 1],
                        )

                # V transposes: vT chunks 4,5 -> V_ext natural layout (+ones)
                for sc in range(N_KCH):
                    base = sc * VEXT_W
                    # ones columns (written once; V cols filled below)
                    nc.any.memset(
                        v_ext[:, base : base + VEXT_W].rearrange(
                            "p (h c) -> p h c", h=HEADS_PER_CORE
                        )[:, :, HD : HD + 1],
                        1.0,
                    )
                    for hp in range(2):  # head pairs
                        tp = tr_ps.tile([128, 128], mdt, tag="trps", name=f"trps_{sc}_{hp}")
                        nc.tensor.transpose(
                            tp[:],
                            qkvT[:, (4 + hp) * S + sc * 128 : (4 + hp) * S + (sc + 1) * 128],
                            ident_sb[:],
                        )
                        nc.vector.tensor_copy(
                            v_ext[:, base + hp * 2 * (HD + 1) : base + (hp * 2 + 2) * (HD + 1)]
                            .rearrange("p (h c) -> p h c", h=2)[:, :, 0:HD],
                            tp[:].rearrange("p (h c) -> p h c", h=2),
                        )

            # ==================== Phase B: attention ====================
            with (
                tc.tile_pool(name="sc_ps", bufs=1, space="PSUM") as sc_ps,
                tc.tile_pool(name="av_ps", bufs=2, space="PSUM") as av_ps,
                tc.tile_pool(name="rep_ps", bufs=1, space="PSUM") as rep_ps,
                tc.tile_pool(name="out_ps", bufs=1, space="PSUM") as out_ps,
                tc.tile_pool(name="pt", bufs=4) as ptp,
                tc.tile_pool(name="den", bufs=2) as denp,
                tc.tile_pool(name="recip", bufs=2) as recipp,
                tc.tile_pool(name="outsb", bufs=3) as outsb,
            ):
                for h in range(HEADS_PER_CORE):
                    qrow = (h % 2) * 64
                    qch = h // 2
                    kch = 2 + h // 2
                    pts = []
                    # ---- scoresT + exp per k-chunk ----
                    for j in range(N_KCH):
                        tj = j // 4
                        W = S - 512 * tj
                        sps = sc_ps.tile([128, 2048], F32, tag="scps", name=f"scps_{h}_{j}")
                        for t in range(tj, 4):
                            nc.tensor.matmul(
                                sps[:, (t - tj) * 512 : (t - tj + 1) * 512],
                                (qkvT[qrow : qrow + 64, kch * S + j * 128 : kch * S + (j + 1) * 128]),
                                (qkvT[qrow : qrow + 64, qch * S + t * 512 : qch * S + (t + 1) * 512]),
                                start=True,
                                stop=True,
                            )
                        db = j * 128 - 512 * tj  # diag block offset within tile
                        nc.vector.tensor_add(
                            sps[:, db : db + 128], sps[:, db : db + 128], tri_sb[:]
                        )
                        pt = ptp.tile([128, W], pdt, tag=f"pt{W}", name=f"pt_{h}_{j}")
                        if db > 0:
                            nc.any.memset(pt[:, 0:db], 0.0)
                        nc.scalar.activation(
                            pt[:, db:W],
                            sps[:, db:W],
                            AF.Exp,
                            bias=kbias_sb[:, j : j + 1],
                            scale=float(HD) ** -0.5,
                        )
                        pts.append(pt)

                    # ---- AV + normalize per q-tile ----
                    recip_rep = recipp.tile([64, S], F32, tag="recip", name=f"recip_{h}")
                    for t in range(4):
                        aps = av_ps.tile([65, 512], F32, tag="avps", name=f"avps_{h}_{t}")
                        jmax = 4 * t + 3
                        for j in range(jmax + 1):
                            tj = j // 4
                            nc.tensor.matmul(
                                aps[:],
                                v_ext[:, j * VEXT_W + h * (HD + 1) : j * VEXT_W + (h + 1) * (HD + 1)],
                                pts[j][:, (t - tj) * 512 : (t - tj + 1) * 512],
                                start=(j == 0),
                                stop=(j == jmax),
                            )
                        den = denp.tile([1, 512], mdt, tag="den", name=f"den_{h}_{t}")
                        nc.vector.tensor_scalar_add(den[:], aps[64:65, :], EPS)
                        rps = rep_ps.tile([64, 512], F32, tag="repps", name=f"repps_{h}_{t}")
                        nc.tensor.matmul(
                            rps[:], (ones_sb[:]), (den[:]), start=True, stop=True
                        )
                        nc.vector.reciprocal(
                            recip_rep[:, t * 512 : (t + 1) * 512], rps[:]
                        )
                        nc.vector.tensor_mul(
                            att_n[qrow : qrow + 64, qch * S + t * 512 : qch * S + (t + 1) * 512],
                            aps[0:64, :],
                            recip_rep[:, t * 512 : (t + 1) * 512],
                        )

                # query-padding mask (same for all heads)
                for ch in range(2):
                    nc.vector.tensor_mul(
                        att_n[:, ch * S : (ch + 1) * S],
                        att_n[:, ch * S : (ch + 1) * S],
                        qmask_sb[:],
                    )

                # ==================== Phase C: out-projection ====================
                for st in range(N_KCH):
                    for n in range(2):
                        ops = out_ps.tile([128, 512], F32, tag="outps", name=f"outps_{st}_{n}")
                        for ch in range(2):
                            nc.tensor.matmul(
                                ops[:],
                                (att_n[:, ch * S + st * 128 : ch * S + (st + 1) * 128]),
                                (wout_sb[:, ch * D + n * 512 : ch * D + (n + 1) * 512]),
                                start=(ch == 0),
                                stop=(ch == 1),
                            )
                        osb = outsb.tile([128, 512], F32, tag="outsb", name=f"outsb_{st}_{n}")
                        nc.any.tensor_copy(osb[:], ops[:])
                        nc.sync.dma_start(
                            out[st * 128 : (st + 1) * 128, n * 512 : (n + 1) * 512],
                            osb[:],
                        )

    return _split_waits(nc) if split_waits else nc


def make_in_maps(x, attention_mask, Wqkv, bqkv, Wout, mm_dt="f32r"):
    """Shard full inputs into the 8 per-core input dicts."""
    rnd = round_f32r if mm_dt == "f32r" else (lambda a: np.ascontiguousarray(a, np.float32))
    x = np.asarray(x, np.float32)
    attention_mask = np.asarray(attention_mask)
    Wqkv = np.asarray(Wqkv, np.float32)
    bqkv = np.asarray(bqkv, np.float32)
    Wout = np.asarray(Wout, np.float32)

    tri = np.where(
        np.arange(128)[:, None] <= np.arange(128)[None, :], 0.0, NEG
    ).astype(np.float32)
    ones64 = np.ones((1, 64), np.float32)
    ident = np.eye(128, dtype=np.float32)

    in_maps = []
    for c in range(CORES):
        b, g = divmod(c, 4)
        cs = 256 * g  # local col start within each of q/k/v blocks
        wq = Wqkv[:, cs : cs + 256]
        wk = Wqkv[:, D + cs : D + cs + 256]
        wv = Wqkv[:, 2 * D + cs : 2 * D + cs + 256]
        w_local = np.ascontiguousarray(np.concatenate([wq, wk, wv], axis=1))
        b_local = np.concatenate(
            [bqkv[cs : cs + 256], bqkv[D + cs : D + cs + 256], bqkv[2 * D + cs : 2 * D + cs + 256]]
        )
        bqkv_pc = np.ascontiguousarray(b_local.reshape(6, 128).T)
        wout_l = np.ascontiguousarray(Wout[cs : cs + 256, :])
        m = attention_mask[b].astype(np.float32)
        kb = np.where(m > 0, 0.0, NEG).astype(np.float32)
        kbias_pc = np.ascontiguousarray(kb.reshape(N_KCH, 128).T)
        qmask_rep = np.ascontiguousarray(np.broadcast_to(m[None, :], (128, S)))
        in_maps.append(
            {
                "xT": rnd(x[b].T),
                "wqkv": rnd(w_local),
                "bqkv_pc": bqkv_pc,
                "wout": rnd(wout_l),
                "kbias": kbias_pc,
                "qmask_rep": qmask_rep,
                "tri": tri,
                "ones64": ones64,  # exact in f32r
                "ident": ident,
            }
        )
    return in_maps


_NC_CACHE = {}


def _get_nc(mm_dt="f32r", p_dt="bf16"):
    key = (mm_dt, p_dt)
    if key not in _NC_CACHE:
        _NC_CACHE[key] = build_nc(*key)
    return _NC_CACHE[key]


def kernel(x, attention_mask, Wqkv, bqkv, Wout, bout, _trace=False, _trace_kwargs=None):
    bout = np.asarray(bout, np.float32)
    mm_dt = os.environ.get("ATTN_MM_DT", "f32r")
    p_dt = os.environ.get("ATTN_P_DT", "bf16")
    in_maps = make_in_maps(x, attention_mask, Wqkv, bqkv, Wout, mm_dt=mm_dt)
    nc = _get_nc(mm_dt, p_dt)
    res = run_bass_kernel_spmd(
        nc,
        in_maps,
        list(range(CORES)),
        trace=_trace,
        **(_trace_kwargs or {}),
    )
    outs = [res.results[c]["out"] for c in range(CORES)]
    full = np.empty((B, S, D), np.float32)
    for b in range(B):
        full[b] = outs[4 * b] + outs[4 * b + 1] + outs[4 * b + 2] + outs[4 * b + 3] + bout
    if _trace:
        return full, res
    return full


# revision 18
# speedup vs baseline: 1.2536x; 1.2536x over previous
"""Trainium2 Bass kernel for causal self-attention (B=2, S=2048, D=1024, H=16).

Sharding: 8 cores = 2 batch groups x 4 head-groups (tensor parallel).
Core c handles batch b = c // 4 and heads [4*(c%4), 4*(c%4)+4).
Each core computes a partial out-projection [S, D]; the host sums the 4
partials of each batch group (row-parallel TP unshard) and adds bout.

Per-core pipeline (all layouts chosen so no on-device transposes of
activations are needed except small V blocks):
  1. qkvT[col, s] = Wqkv_local.T @ x.T   (x passed pre-transposed, a host
     layout choice; weights are naturally [D, cols] = lhsT layout)
  2. scoresT[k, q] = K_h^T.T @ Q_h per 128-wide k-chunk, causal blocks only.
     Key-padding mask + 1/sqrt(64) scale fold into the ACT exp (per-partition
     bias = per-k bias in this transposed layout).  P = exp(scores') in bf16.
  3. attT[65, q] = V_ext^T @ P  where V_ext = [V_h | ones]: row 64 is the
     softmax denominator.  No separate reduction needed.
  4. normalize per-q: recip = 1/(den + eps) replicated via a K=1 matmul;
     att_n = attT * recip; query-padding mask applied as one big multiply.
  5. out_partial[s, :] = att_n.T @ Wout_local  (att_n is already the lhsT
     layout needed), DMA PSUM -> DRAM directly.
"""

import os
import sys

import numpy as np

for _p in ("/opt/trn_rl_repo",):
    if _p not in sys.path and os.path.isdir(_p):
        sys.path.insert(0, _p)

import concourse.bass as bass
import concourse.mybir as mybir
from concourse import tile
from concourse.bass_utils import run_bass_kernel_spmd

B, S, D, H = 2, 2048, 1024, 16
HD = D // H  # 64
HEADS_PER_CORE = 4
CORES = 8
LOCAL_COLS = 3 * HEADS_PER_CORE * HD  # 768 (q|k|v for 4 heads)
NEG = -1.0e30
EPS = 1.0e-9  # within ACT-reciprocal valid range +-[2^-42, 2^42]

F32 = mybir.dt.float32
F32R = mybir.dt.float32r
BF16 = mybir.dt.bfloat16

AF = mybir.ActivationFunctionType

N_STILE = 4  # 512-wide s tiles
N_KCH = S // 128  # 16 k-chunks
VEXT_W = HEADS_PER_CORE * (HD + 1)  # 260


def round_f32r(a):
    """Round fp32 array to fp32r (11-bit mantissa, round-to-nearest-even)."""
    u = np.ascontiguousarray(a, np.float32).view(np.uint32)
    low = u & np.uint32(0x00000FFF)
    base = u & np.uint32(0xFFFFF000)
    lsb = (u >> np.uint32(12)) & np.uint32(1)
    up = (low > 0x800) | ((low == 0x800) & (lsb == 1))
    return (base + (up.astype(np.uint32) << np.uint32(12))).view(np.float32)



def _split_waits(nc, cap=1):
    """Walrus in this container allows few sync-waits per instruction.
    Hoist excess waits onto preceding same-engine NoOps (same sequencer,
    program order => semantics preserved).  fp32-path Matmult lowers to
    LDW+MM whose LW struct takes no waits at all -> cap 0."""
    uid = [0]
    for fn in nc.m.functions:
        for bb in fn.blocks:
            insts = bb.instructions
            out = []
            for ins in insts:
                icap = 0 if isinstance(ins, mybir.InstMatmult) else cap
                si = ins.sync_info
                waits = list(si.on_wait) if (si and si.on_wait) else []
                if len(waits) > icap:
                    extra = waits[:-icap] if icap else waits
                    keep = waits[-icap:] if icap else []
                    gcap = max(cap, 1)
                    for i in range(0, len(extra), gcap):
                        grp = extra[i : i + gcap]
                        nop = mybir.InstNoOp(
                            name=f"wsplit-{uid[0]}", ins=[], outs=[]
                        )
                        uid[0] += 1
                        nop.engine = ins.engine
                        nop.sync_info = mybir.SyncInfo(on_wait=grp, on_update=[])
                        out.append(nop)
                    si.on_wait = keep
                out.append(ins)
            if len(out) != len(insts):
                insts[:] = out
    return nc


# score-chunk table: per tj (= j//4), list of (start_col, width) chunks of
# the valid q-range [512*tj, 2048), each <= 1024 wide, 512-aligned pieces
CHUNKS = {
    0: [(0, 1024), (1024, 1024)],
    1: [(512, 512), (1024, 1024)],
    2: [(1024, 1024)],
    3: [(1536, 512)],
}


def _chunk_for(tj, col):
    for cs, cw in CHUNKS[tj]:
        if cs <= col < cs + cw:
            return cs, cw
    raise ValueError((tj, col))


def _act_recip(nc, out_ap, in_ap):
    """ACT-engine reciprocal (bass blocks ActivationFunctionType.Reciprocal
    behind an accuracy warning; ~1e-5 rel err is fine for this kernel and it
    replaces a 53us DVE InstReciprocal with one ~2us ACTIVATE)."""
    eng = nc.scalar
    inputs = [eng.lower_ap(in_ap)]
    for v in (0.0, 1.0, 0.0):  # bias, scale, alpha
        inputs.append(mybir.ImmediateValue(dtype=mybir.dt.float32, value=v))
    return eng.add_instruction(
        mybir.InstActivation(
            name=eng.bass.get_next_instruction_name(),
            func=mybir.ActivationFunctionType.Reciprocal,
            ins=inputs,
            outs=[eng.lower_ap(out_ap)],
        )
    )


def build_nc(mm_dt="f32r", p_dt="bf16", split_waits=True):
    """Build the SPMD single-core program (same program on all 8 cores)."""
    nc = bass.Bass()
    mdt = F32R if mm_dt == "f32r" else F32
    pdt = BF16 if p_dt == "bf16" else F32
    scale = float(HD) ** -0.5

    xT = nc.dram_tensor("xT", [D, S], mdt, kind="ExternalInput")
    wqkv = nc.dram_tensor("wqkv", [D, LOCAL_COLS], mdt, kind="ExternalInput")
    bqkv_pc = nc.dram_tensor("bqkv_pc", [128, 6], F32, kind="ExternalInput")
    wout = nc.dram_tensor("wout", [256, D], mdt, kind="ExternalInput")
    kbias = nc.dram_tensor("kbias", [128, N_KCH], F32, kind="ExternalInput")
    qmask_rep = nc.dram_tensor("qmask_rep", [128, S], F32, kind="ExternalInput")
    tri = nc.dram_tensor("tri", [128, 128], F32, kind="ExternalInput")
    ident = nc.dram_tensor("ident", [128, 128], pdt, kind="ExternalInput")
    out = nc.dram_tensor("out", [S, D], F32, kind="ExternalOutput")

    with tile.TileContext(nc) as tc:
        with (
            tc.tile_pool(name="consts", bufs=1) as consts,
            tc.tile_pool(name="persist", bufs=1) as persist,
        ):
            # ---- constants / persistent SBUF ----
            wqkv_sb = consts.tile([128, 8 * LOCAL_COLS], mdt)  # 8 d-chunks
            for d in range(8):
                nc.sync.dma_start(
                    wqkv_sb[:, d * LOCAL_COLS : (d + 1) * LOCAL_COLS],
                    wqkv[d * 128 : (d + 1) * 128, :],
                )
            bqkv_sb = consts.tile([128, 6], F32)
            nc.sync.dma_start(bqkv_sb[:], bqkv_pc[:])
            wout_sb = consts.tile([128, 2 * D], mdt)
            for ch in range(2):
                nc.sync.dma_start(
                    wout_sb[:, ch * D : (ch + 1) * D],
                    wout[ch * 128 : (ch + 1) * 128, :],
                )
            kbias_sb = consts.tile([128, N_KCH], F32)
            nc.sync.dma_start(kbias_sb[:], kbias[:])
            qmask_sb = consts.tile([128, S], F32)
            nc.sync.dma_start(qmask_sb[:], qmask_rep[:])
            tri_sb = consts.tile([128, 128], F32)
            nc.sync.dma_start(tri_sb[:], tri[:])
            ident_sb = consts.tile([128, 128], pdt)
            nc.sync.dma_start(ident_sb[:], ident[:])

            # qkvT: 6 col-chunks x [128, S] in bf16; 0,1 = q, 2,3 = k, 4,5 = v
            qkvT = persist.tile([128, 6 * S], pdt)
            # V_ext: per k-chunk [128, 260]: 4 heads x (64 V cols + ones col)
            v_ext = persist.tile([128, N_KCH * VEXT_W], pdt)
            # att_u: attended (transposed), unnormalized then normalized in place
            att_u = persist.tile([128, 2 * S], mdt)
            # denominators: one row per head at partition h*32 (engine start-
            # partition constraint: must be 0/32/64/96)
            den4 = persist.tile([128, S], F32)
            recip4 = persist.tile([128, S], F32)

            # ==================== Phase A: QKV ====================
            with (
                tc.tile_pool(name="xs", bufs=3) as xs,
                tc.tile_pool(name="qkv_ps", bufs=6, space="PSUM") as qkv_ps,
                tc.tile_pool(name="tr_ps", bufs=2, space="PSUM") as tr_ps,
            ):
                for t in range(N_STILE):
                    ps = [qkv_ps.tile([128, 512], F32, tag="qkvps", name=f"qkvps_{t}_{i}") for i in range(6)]
                    for d in range(8):
                        xt = xs.tile([128, 512], mdt, tag="xs", name=f"xs_{t}_{d}")
                        nc.gpsimd.dma_start(
                            xt[:], xT[d * 128 : (d + 1) * 128, t * 512 : (t + 1) * 512]
                        )
                        for cc in range(6):
                            nc.tensor.matmul(
                                ps[cc][:],
                                wqkv_sb[:, d * LOCAL_COLS + cc * 128 : d * LOCAL_COLS + (cc + 1) * 128],
                                xt[:],
                                start=(d == 0),
                                stop=(d == 7),
                            )
                    for cc in range(6):
                        nc.scalar.activation(
                            qkvT[:, cc * S + t * 512 : cc * S + (t + 1) * 512],
                            ps[cc][:],
                            AF.Identity,
                            bias=bqkv_sb[:, cc : cc + 1],
                        )

                # V transposes: vT chunks 4,5 -> V_ext natural layout (+ones)
                for sc in range(N_KCH):
                    base = sc * VEXT_W
                    nc.any.memset(
                        v_ext[:, base : base + VEXT_W].rearrange(
                            "p (h c) -> p h c", h=HEADS_PER_CORE
                        )[:, :, HD : HD + 1],
                        1.0,
                    )
                    for hp in range(2):  # head pairs
                        tp = tr_ps.tile([128, 128], pdt, tag="trps", name=f"trps_{sc}_{hp}")
                        nc.tensor.transpose(
                            tp[:],
                            qkvT[:, (4 + hp) * S + sc * 128 : (4 + hp) * S + (sc + 1) * 128],
                            ident_sb[:],
                        )
                        nc.vector.tensor_copy(
                            v_ext[:, base + hp * 2 * (HD + 1) : base + (hp * 2 + 2) * (HD + 1)]
                            .rearrange("p (h c) -> p h c", h=2)[:, :, 0:HD],
                            tp[:].rearrange("p (h c) -> p h c", h=2),
                        )

            # ==================== Phase B: attention ====================
            with (
                tc.tile_pool(name="sc_ps", bufs=2, space="PSUM") as sc_ps,
                tc.tile_pool(name="av_ps", bufs=2, space="PSUM") as av_ps,
                tc.tile_pool(name="out_ps", bufs=2, space="PSUM") as out_ps,
                tc.tile_pool(name="pt", bufs=4) as ptp,
                tc.tile_pool(name="rr", bufs=3) as rrp,
                tc.tile_pool(name="outsb", bufs=2) as outsb,
                tc.tile_pool(name="dram", bufs=1, space="DRAM") as dramp,
            ):
                recip4_dram = dramp.tile([4, S], F32, name="recip4_dram")
                for h in range(HEADS_PER_CORE):
                    qrow = (h % 2) * 64
                    qch = h // 2
                    kch = 2 + h // 2
                    pts = {}
                    # ---- scoresT + exp, chunked <=1024 wide ----
                    for j in range(N_KCH):
                        tj = j // 4
                        for ci, (cs, cw) in enumerate(CHUNKS[tj]):
                            sps = sc_ps.tile(
                                [128, 1024], F32, tag="scps", name=f"scps_{h}_{j}_{ci}"
                            )
                            for o in range(0, cw, 512):
                                t = (cs + o) // 512
                                nc.tensor.matmul(
                                    sps[:, o : o + 512],
                                    qkvT[qrow : qrow + 64, kch * S + j * 128 : kch * S + (j + 1) * 128],
                                    qkvT[qrow : qrow + 64, qch * S + t * 512 : qch * S + (t + 1) * 512],
                                    start=True,
                                    stop=True,
                                )
                            pt = ptp.tile(
                                [128, cw], pdt, tag=f"pt{cw}",
                                bufs=(18 if cw == 1024 else 10),
                                name=f"pt_{h}_{j}_{ci}",
                            )
                            if ci == 0:
                                db = j * 128 - cs  # diag block offset in chunk
                                nc.vector.tensor_add(
                                    sps[:, db : db + 128], sps[:, db : db + 128], tri_sb[:]
                                )
                                if db > 0:
                                    nc.any.memset(pt[:, 0:db], 0.0)
                                nc.scalar.activation(
                                    pt[:, db:cw], sps[:, db:cw], AF.Exp,
                                    bias=kbias_sb[:, j : j + 1], scale=scale,
                                )
                            else:
                                nc.scalar.activation(
                                    pt[:, 0:cw], sps[:, 0:cw], AF.Exp,
                                    bias=kbias_sb[:, j : j + 1], scale=scale,
                                )
                            pts[(j, cs)] = pt

                    # ---- AV per q-tile; denominator row + unnormalized copy ----
                    for t in range(4):
                        aps = av_ps.tile([65, 512], F32, tag="avps", name=f"avps_{h}_{t}")
                        jmax = 4 * t + 3
                        for j in range(jmax + 1):
                            tj = j // 4
                            cs, cw = _chunk_for(tj, t * 512)
                            nc.tensor.matmul(
                                aps[:],
                                v_ext[:, j * VEXT_W + h * (HD + 1) : j * VEXT_W + (h + 1) * (HD + 1)],
                                pts[(j, cs)][:, t * 512 - cs : t * 512 - cs + 512],
                                start=(j == 0),
                                stop=(j == jmax),
                            )
                        nc.vector.tensor_scalar_add(
                            den4[h * 32 : h * 32 + 1, t * 512 : (t + 1) * 512],
                            aps[64:65, :],
                            EPS,
                        )
                        nc.scalar.activation(
                            att_u[qrow : qrow + 64, qch * S + t * 512 : qch * S + (t + 1) * 512],
                            aps[0:64, :],
                            AF.Identity,
                        )

                # ---- batched reciprocals (back-to-back: one table switch) ----
                for h in range(HEADS_PER_CORE):
                    _act_recip(
                        nc,
                        recip4[h * 32 : h * 32 + 1, :],
                        den4[h * 32 : h * 32 + 1, :],
                    )
                nc.sync.dma_start(
                    recip4_dram[:],
                    recip4[:].rearrange("(a b) c -> a b c", b=32)[:, 0:1, :].rearrange(
                        "a b c -> (a b) c"
                    ),
                )

                # ---- normalize in place: att_u *= broadcast(recip4[h]) ----
                # one [128, 512] rr tile serves a head PAIR (same qkvT chunk)
                for qch in range(2):
                    for t in range(4):
                        rr = rrp.tile([128, 512], F32, tag="rr", name=f"rr_{qch}_{t}")
                        for hh in range(2):
                            h = qch * 2 + hh
                            nc.sync.dma_start(
                                rr[hh * 64 : (hh + 1) * 64, :],
                                recip4_dram[h : h + 1, t * 512 : (t + 1) * 512].to_broadcast((64, 512)),
                            )
                        sl = att_u[:, qch * S + t * 512 : qch * S + (t + 1) * 512]
                        nc.vector.tensor_mul(sl, sl, rr[:])

                # query-padding mask (same for all heads)
                for ch in range(2):
                    nc.vector.tensor_mul(
                        att_u[:, ch * S : (ch + 1) * S],
                        att_u[:, ch * S : (ch + 1) * S],
                        qmask_sb[:],
                    )

                # ==================== Phase C: out-projection ====================
                for st in range(N_KCH):
                    for n in range(2):
                        ops = out_ps.tile([128, 512], F32, tag="outps", name=f"outps_{st}_{n}")
                        for ch in range(2):
                            nc.tensor.matmul(
                                ops[:],
                                att_u[:, ch * S + st * 128 : ch * S + (st + 1) * 128],
                                wout_sb[:, ch * D + n * 512 : ch * D + (n + 1) * 512],
                                start=(ch == 0),
                                stop=(ch == 1),
                            )
                        osb = outsb.tile([128, 512], F32, tag="outsb", name=f"outsb_{st}_{n}")
                        nc.vector.tensor_copy(osb[:], ops[:])
                        nc.sync.dma_start(
                            out[st * 128 : (st + 1) * 128, n * 512 : (n + 1) * 512],
                            osb[:],
                        )

    return _split_waits(nc) if split_waits else nc


def make_in_maps(x, attention_mask, Wqkv, bqkv, Wout, mm_dt="f32r"):
    """Shard full inputs into the 8 per-core input dicts."""
    rnd = round_f32r if mm_dt == "f32r" else (lambda a: np.ascontiguousarray(a, np.float32))
    x = np.asarray(x, np.float32)
    attention_mask = np.asarray(attention_mask)
    Wqkv = np.asarray(Wqkv, np.float32)
    bqkv = np.asarray(bqkv, np.float32)
    Wout = np.asarray(Wout, np.float32)

    import ml_dtypes

    tri = np.where(
        np.arange(128)[:, None] <= np.arange(128)[None, :], 0.0, NEG
    ).astype(np.float32)
    ident = np.eye(128, dtype=ml_dtypes.bfloat16)

    in_maps = []
    for c in range(CORES):
        b, g = divmod(c, 4)
        cs = 256 * g  # local col start within each of q/k/v blocks
        wq = Wqkv[:, cs : cs + 256]
        wk = Wqkv[:, D + cs : D + cs + 256]
        wv = Wqkv[:, 2 * D + cs : 2 * D + cs + 256]
        w_local = np.ascontiguousarray(np.concatenate([wq, wk, wv], axis=1))
        b_local = np.concatenate(
            [bqkv[cs : cs + 256], bqkv[D + cs : D + cs + 256], bqkv[2 * D + cs : 2 * D + cs + 256]]
        )
        bqkv_pc = np.ascontiguousarray(b_local.reshape(6, 128).T)
        wout_l = np.ascontiguousarray(Wout[cs : cs + 256, :])
        m = attention_mask[b].astype(np.float32)
        kb = np.where(m > 0, 0.0, NEG).astype(np.float32)
        kbias_pc = np.ascontiguousarray(kb.reshape(N_KCH, 128).T)
        qmask_rep = np.ascontiguousarray(np.broadcast_to(m[None, :], (128, S)))
        in_maps.append(
            {
                "xT": rnd(x[b].T),
                "wqkv": rnd(w_local),
                "bqkv_pc": bqkv_pc,
                "wout": rnd(wout_l),
                "kbias": kbias_pc,
                "qmask_rep": qmask_rep,
                "tri": tri,
                "ident": ident,
            }
        )
    return in_maps


_NC_CACHE = {}


def _get_nc(mm_dt="f32r", p_dt="bf16"):
    key = (mm_dt, p_dt)
    if key not in _NC_CACHE:
        _NC_CACHE[key] = build_nc(*key)
    return _NC_CACHE[key]


def kernel(x, attention_mask, Wqkv, bqkv, Wout, bout, _trace=False, _trace_kwargs=None):
    bout = np.asarray(bout, np.float32)
    mm_dt = os.environ.get("ATTN_MM_DT", "f32r")
    p_dt = os.environ.get("ATTN_P_DT", "bf16")
    in_maps = make_in_maps(x, attention_mask, Wqkv, bqkv, Wout, mm_dt=mm_dt)
    nc = _get_nc(mm_dt, p_dt)
    res = run_bass_kernel_spmd(
        nc,
        in_maps,
        list(range(CORES)),
        trace=_trace,
        **(_trace_kwargs or {}),
    )
    outs = [res.results[c]["out"] for c in range(CORES)]
    full = np.empty((B, S, D), np.float32)
    for b in range(B):
        full[b] = outs[4 * b] + outs[4 * b + 1] + outs[4 * b + 2] + outs[4 * b + 3] + bout
    if _trace:
        return full, res
    return full


# revision 19
# speedup vs baseline: 1.2732x; 1.0157x over previous
"""Trainium2 Bass kernel for causal self-attention (B=2, S=2048, D=1024, H=16).

Sharding: 8 cores = 2 batch groups x 4 head-groups (tensor parallel).
Core c handles batch b = c // 4 and heads [4*(c%4), 4*(c%4)+4).
Each core computes a partial out-projection [S, D]; the host sums the 4
partials of each batch group (row-parallel TP unshard) and adds bout.

Per-core pipeline (all layouts chosen so no on-device transposes of
activations are needed except small V blocks):
  1. qkvT[col, s] = Wqkv_local.T @ x.T   (x passed pre-transposed, a host
     layout choice; weights are naturally [D, cols] = lhsT layout)
  2. scoresT[k, q] = K_h^T.T @ Q_h per 128-wide k-chunk, causal blocks only.
     Key-padding mask + 1/sqrt(64) scale fold into the ACT exp (per-partition
     bias = per-k bias in this transposed layout).  P = exp(scores') in bf16.
  3. attT[65, q] = V_ext^T @ P  where V_ext = [V_h | ones]: row 64 is the
     softmax denominator.  No separate reduction needed.
  4. normalize per-q: recip = 1/(den + eps) replicated via a K=1 matmul;
     att_n = attT * recip; query-padding mask applied as one big multiply.
  5. out_partial[s, :] = att_n.T @ Wout_local  (att_n is already the lhsT
     layout needed), DMA PSUM -> DRAM directly.
"""

import os
import sys

import numpy as np

for _p in ("/opt/trn_rl_repo",):
    if _p not in sys.path and os.path.isdir(_p):
        sys.path.insert(0, _p)

import concourse.bass as bass
import concourse.mybir as mybir
from concourse import tile
from concourse.bass_utils import run_bass_kernel_spmd

B, S, D, H = 2, 2048, 1024, 16
HD = D // H  # 64
HEADS_PER_CORE = 4
CORES = 8
LOCAL_COLS = 3 * HEADS_PER_CORE * HD  # 768 (q|k|v for 4 heads)
NEG = -1.0e30
EPS = 1.0e-9  # within ACT-reciprocal valid range +-[2^-42, 2^42]

F32 = mybir.dt.float32
F32R = mybir.dt.float32r
BF16 = mybir.dt.bfloat16

AF = mybir.ActivationFunctionType

N_STILE = 4  # 512-wide s tiles
N_KCH = S // 128  # 16 k-chunks
VEXT_W = HEADS_PER_CORE * (HD + 1)  # 260


def round_f32r(a):
    """Round fp32 array to fp32r (11-bit mantissa, round-to-nearest-even)."""
    u = np.ascontiguousarray(a, np.float32).view(np.uint32)
    low = u & np.uint32(0x00000FFF)
    base = u & np.uint32(0xFFFFF000)
    lsb = (u >> np.uint32(12)) & np.uint32(1)
    up = (low > 0x800) | ((low == 0x800) & (lsb == 1))
    return (base + (up.astype(np.uint32) << np.uint32(12))).view(np.float32)



def _split_waits(nc, cap=1):
    """Walrus in this container allows few sync-waits per instruction.
    Hoist excess waits onto preceding same-engine NoOps (same sequencer,
    program order => semantics preserved).  fp32-path Matmult lowers to
    LDW+MM whose LW struct takes no waits at all -> cap 0."""
    uid = [0]
    for fn in nc.m.functions:
        for bb in fn.blocks:
            insts = bb.instructions
            out = []
            for ins in insts:
                icap = 0 if isinstance(ins, mybir.InstMatmult) else cap
                si = ins.sync_info
                waits = list(si.on_wait) if (si and si.on_wait) else []
                if len(waits) > icap:
                    extra = waits[:-icap] if icap else waits
                    keep = waits[-icap:] if icap else []
                    gcap = max(cap, 1)
                    for i in range(0, len(extra), gcap):
                        grp = extra[i : i + gcap]
                        nop = mybir.InstNoOp(
                            name=f"wsplit-{uid[0]}", ins=[], outs=[]
                        )
                        uid[0] += 1
                        nop.engine = ins.engine
                        nop.sync_info = mybir.SyncInfo(on_wait=grp, on_update=[])
                        out.append(nop)
                    si.on_wait = keep
                out.append(ins)
            if len(out) != len(insts):
                insts[:] = out
    return nc


# score-chunk table: per tj (= j//4), list of (start_col, width) chunks of
# the valid q-range [512*tj, 2048), each <= 1024 wide, 512-aligned pieces
CHUNKS = {
    0: [(0, 1024), (1024, 1024)],
    1: [(512, 512), (1024, 1024)],
    2: [(1024, 1024)],
    3: [(1536, 512)],
}


def _chunk_for(tj, col):
    for cs, cw in CHUNKS[tj]:
        if cs <= col < cs + cw:
            return cs, cw
    raise ValueError((tj, col))


def _act_recip(nc, out_ap, in_ap):
    """ACT-engine reciprocal (bass blocks ActivationFunctionType.Reciprocal
    behind an accuracy warning; ~1e-5 rel err is fine for this kernel and it
    replaces a 53us DVE InstReciprocal with one ~2us ACTIVATE)."""
    eng = nc.scalar
    inputs = [eng.lower_ap(in_ap)]
    for v in (0.0, 1.0, 0.0):  # bias, scale, alpha
        inputs.append(mybir.ImmediateValue(dtype=mybir.dt.float32, value=v))
    return eng.add_instruction(
        mybir.InstActivation(
            name=eng.bass.get_next_instruction_name(),
            func=mybir.ActivationFunctionType.Reciprocal,
            ins=inputs,
            outs=[eng.lower_ap(out_ap)],
        )
    )


def build_nc(mm_dt="f32r", p_dt="bf16", split_waits=True):
    """Build the SPMD single-core program (same program on all 8 cores)."""
    nc = bass.Bass()
    mdt = F32R if mm_dt == "f32r" else F32
    pdt = BF16 if p_dt == "bf16" else F32
    scale = float(HD) ** -0.5

    xT = nc.dram_tensor("xT", [D, S], mdt, kind="ExternalInput")
    wqkv = nc.dram_tensor("wqkv", [D, LOCAL_COLS], mdt, kind="ExternalInput")
    bqkv_pc = nc.dram_tensor("bqkv_pc", [128, 6], F32, kind="ExternalInput")
    wout = nc.dram_tensor("wout", [256, D], mdt, kind="ExternalInput")
    kbias = nc.dram_tensor("kbias", [128, N_KCH], F32, kind="ExternalInput")
    qmask_rep = nc.dram_tensor("qmask_rep", [128, S], F32, kind="ExternalInput")
    tri = nc.dram_tensor("tri", [128, 128], F32, kind="ExternalInput")
    ident = nc.dram_tensor("ident", [128, 128], pdt, kind="ExternalInput")
    out = nc.dram_tensor("out", [S, D], F32, kind="ExternalOutput")

    with tile.TileContext(nc) as tc:
        with (
            tc.tile_pool(name="consts", bufs=1) as consts,
            tc.tile_pool(name="persist", bufs=1) as persist,
        ):
            # ---- constants / persistent SBUF ----
            wout_sb = consts.tile([128, 2 * D], mdt)
            for ch in range(2):
                nc.sync.dma_start(
                    wout_sb[:, ch * D : (ch + 1) * D],
                    wout[ch * 128 : (ch + 1) * 128, :],
                )
            kbias_sb = consts.tile([128, N_KCH], F32)
            nc.sync.dma_start(kbias_sb[:], kbias[:])
            qmask_sb = consts.tile([128, S], F32)
            nc.sync.dma_start(qmask_sb[:], qmask_rep[:])
            tri_sb = consts.tile([128, 128], F32)
            nc.sync.dma_start(tri_sb[:], tri[:])

            # qkvT: 6 col-chunks x [128, S] in bf16; 0,1 = q, 2,3 = k, 4,5 = v
            qkvT = persist.tile([128, 6 * S], pdt)
            # V_ext: per k-chunk [128, 260]: 4 heads x (64 V cols + ones col)
            v_ext = persist.tile([128, N_KCH * VEXT_W], pdt)
            # att_u: attended (transposed), unnormalized then normalized in place
            att_u = persist.tile([128, 2 * S], mdt)
            # denominators: one row per head at partition h*32 (engine start-
            # partition constraint: must be 0/32/64/96)
            den4 = persist.tile([128, S], F32)
            recip4 = persist.tile([128, S], F32)

            # ==================== Phase A: QKV ====================
            with (
                tc.tile_pool(name="aconsts", bufs=1) as aconsts,
                tc.tile_pool(name="xs", bufs=3) as xs,
                tc.tile_pool(name="qkv_ps", bufs=6, space="PSUM") as qkv_ps,
                tc.tile_pool(name="tr_ps", bufs=2, space="PSUM") as tr_ps,
            ):
                wqkv_sb = aconsts.tile([128, 8 * LOCAL_COLS], mdt)
                for d in range(8):
                    nc.sync.dma_start(
                        wqkv_sb[:, d * LOCAL_COLS : (d + 1) * LOCAL_COLS],
                        wqkv[d * 128 : (d + 1) * 128, :],
                    )
                bqkv_sb = aconsts.tile([128, 6], F32)
                nc.sync.dma_start(bqkv_sb[:], bqkv_pc[:])
                ident_sb = aconsts.tile([128, 128], pdt)
                nc.sync.dma_start(ident_sb[:], ident[:])
                for t in range(N_STILE):
                    ps = [qkv_ps.tile([128, 512], F32, tag="qkvps", name=f"qkvps_{t}_{i}") for i in range(6)]
                    for d in range(8):
                        xt = xs.tile([128, 512], mdt, tag="xs", name=f"xs_{t}_{d}")
                        nc.gpsimd.dma_start(
                            xt[:], xT[d * 128 : (d + 1) * 128, t * 512 : (t + 1) * 512]
                        )
                        for cc in range(6):
                            nc.tensor.matmul(
                                ps[cc][:],
                                wqkv_sb[:, d * LOCAL_COLS + cc * 128 : d * LOCAL_COLS + (cc + 1) * 128],
                                xt[:],
                                start=(d == 0),
                                stop=(d == 7),
                            )
                    for cc in range(6):
                        nc.vector.tensor_scalar_add(
                            qkvT[:, cc * S + t * 512 : cc * S + (t + 1) * 512],
                            ps[cc][:],
                            bqkv_sb[:, cc : cc + 1],
                        )

                # V transposes: vT chunks 4,5 -> V_ext natural layout (+ones)
                for sc in range(N_KCH):
                    base = sc * VEXT_W
                    nc.any.memset(
                        v_ext[:, base : base + VEXT_W].rearrange(
                            "p (h c) -> p h c", h=HEADS_PER_CORE
                        )[:, :, HD : HD + 1],
                        1.0,
                    )
                    for hp in range(2):  # head pairs
                        tp = tr_ps.tile([128, 128], pdt, tag="trps", name=f"trps_{sc}_{hp}")
                        nc.tensor.transpose(
                            tp[:],
                            qkvT[:, (4 + hp) * S + sc * 128 : (4 + hp) * S + (sc + 1) * 128],
                            ident_sb[:],
                        )
                        nc.vector.tensor_copy(
                            v_ext[:, base + hp * 2 * (HD + 1) : base + (hp * 2 + 2) * (HD + 1)]
                            .rearrange("p (h c) -> p h c", h=2)[:, :, 0:HD],
                            tp[:].rearrange("p (h c) -> p h c", h=2),
                        )

            # ==================== Phase B: attention ====================
            with (
                tc.tile_pool(name="sc_ps", bufs=2, space="PSUM") as sc_ps,
                tc.tile_pool(name="av_ps", bufs=2, space="PSUM") as av_ps,
                tc.tile_pool(name="out_ps", bufs=2, space="PSUM") as out_ps,
                tc.tile_pool(name="pt", bufs=4) as ptp,
                tc.tile_pool(name="rr", bufs=3) as rrp,
                tc.tile_pool(name="outsb", bufs=2) as outsb,
                tc.tile_pool(name="dram", bufs=1, space="DRAM") as dramp,
            ):
                recip4_dram = dramp.tile([4, S], F32, name="recip4_dram")
                for h in range(HEADS_PER_CORE):
                    qrow = (h % 2) * 64
                    qch = h // 2
                    kch = 2 + h // 2
                    pts = {}
                    # ---- scoresT + exp, chunked <=1024 wide ----
                    for j in range(N_KCH):
                        tj = j // 4
                        for ci, (cs, cw) in enumerate(CHUNKS[tj]):
                            sps = sc_ps.tile(
                                [128, 1024], F32, tag="scps", name=f"scps_{h}_{j}_{ci}"
                            )
                            for o in range(0, cw, 512):
                                t = (cs + o) // 512
                                nc.tensor.matmul(
                                    sps[:, o : o + 512],
                                    qkvT[qrow : qrow + 64, kch * S + j * 128 : kch * S + (j + 1) * 128],
                                    qkvT[qrow : qrow + 64, qch * S + t * 512 : qch * S + (t + 1) * 512],
                                    start=True,
                                    stop=True,
                                )
                            pt = ptp.tile(
                                [128, cw], pdt, tag=f"pt{cw}",
                                bufs=(32 if cw == 1024 else 16),
                                name=f"pt_{h}_{j}_{ci}",
                            )
                            if ci == 0:
                                db = j * 128 - cs  # diag block offset in chunk
                                nc.vector.tensor_add(
                                    sps[:, db : db + 128], sps[:, db : db + 128], tri_sb[:]
                                )
                                if db > 0:
                                    nc.any.memset(pt[:, 0:db], 0.0)
                                nc.scalar.activation(
                                    pt[:, db:cw], sps[:, db:cw], AF.Exp,
                                    bias=kbias_sb[:, j : j + 1], scale=scale,
                                )
                            else:
                                nc.scalar.activation(
                                    pt[:, 0:cw], sps[:, 0:cw], AF.Exp,
                                    bias=kbias_sb[:, j : j + 1], scale=scale,
                                )
                            pts[(j, cs)] = pt

                    # ---- AV per q-tile; denominator row + unnormalized copy ----
                    for t in range(4):
                        aps = av_ps.tile([65, 512], F32, tag="avps", name=f"avps_{h}_{t}")
                        jmax = 4 * t + 3
                        for j in range(jmax + 1):
                            tj = j // 4
                            cs, cw = _chunk_for(tj, t * 512)
                            nc.tensor.matmul(
                                aps[:],
                                v_ext[:, j * VEXT_W + h * (HD + 1) : j * VEXT_W + (h + 1) * (HD + 1)],
                                pts[(j, cs)][:, t * 512 - cs : t * 512 - cs + 512],
                                start=(j == 0),
                                stop=(j == jmax),
                            )
                        nc.vector.tensor_scalar_add(
                            den4[h * 32 : h * 32 + 1, t * 512 : (t + 1) * 512],
                            aps[64:65, :],
                            EPS,
                        )
                        nc.scalar.activation(
                            att_u[qrow : qrow + 64, qch * S + t * 512 : qch * S + (t + 1) * 512],
                            aps[0:64, :],
                            AF.Identity,
                        )

                # ---- batched reciprocals (back-to-back: one table switch),
                # with the query-padding mask folded into the recip rows ----
                for h in range(HEADS_PER_CORE):
                    _act_recip(
                        nc,
                        recip4[h * 32 : h * 32 + 1, :],
                        den4[h * 32 : h * 32 + 1, :],
                    )
                    nc.vector.tensor_mul(
                        recip4[h * 32 : h * 32 + 1, :],
                        recip4[h * 32 : h * 32 + 1, :],
                        qmask_sb[h * 32 : h * 32 + 1, :],
                    )
                nc.sync.dma_start(
                    recip4_dram[:],
                    recip4[:].rearrange("(a b) c -> a b c", b=32)[:, 0:1, :].rearrange(
                        "a b c -> (a b) c"
                    ),
                )

                # ---- normalize in place + out-projection, pipelined per q-tile ----
                for t in range(4):
                    for qch in range(2):
                        rr = rrp.tile([128, 512], F32, tag="rr", name=f"rr_{qch}_{t}")
                        for hh in range(2):
                            h = qch * 2 + hh
                            nc.sync.dma_start(
                                rr[hh * 64 : (hh + 1) * 64, :],
                                recip4_dram[h : h + 1, t * 512 : (t + 1) * 512].to_broadcast((64, 512)),
                            )
                        sl = att_u[:, qch * S + t * 512 : qch * S + (t + 1) * 512]
                        nc.vector.tensor_mul(sl, sl, rr[:])
                    for st in range(4 * t, 4 * t + 4):
                        for n in range(2):
                            ops = out_ps.tile([128, 512], F32, tag="outps", name=f"outps_{st}_{n}")
                            for ch in range(2):
                                nc.tensor.matmul(
                                    ops[:],
                                    att_u[:, ch * S + st * 128 : ch * S + (st + 1) * 128],
                                    wout_sb[:, ch * D + n * 512 : ch * D + (n + 1) * 512],
                                    start=(ch == 0),
                                    stop=(ch == 1),
                                )
                            osb = outsb.tile([128, 512], F32, tag="outsb", name=f"outsb_{st}_{n}")
                            nc.vector.tensor_copy(osb[:], ops[:])
                            nc.sync.dma_start(
                                out[st * 128 : (st + 1) * 128, n * 512 : (n + 1) * 512],
                                osb[:],
                            )

    return _split_waits(nc) if split_waits else nc


def make_in_maps(x, attention_mask, Wqkv, bqkv, Wout, mm_dt="f32r"):
    """Shard full inputs into the 8 per-core input dicts."""
    rnd = round_f32r if mm_dt == "f32r" else (lambda a: np.ascontiguousarray(a, np.float32))
    x = np.asarray(x, np.float32)
    attention_mask = np.asarray(attention_mask)
    Wqkv = np.asarray(Wqkv, np.float32)
    bqkv = np.asarray(bqkv, np.float32)
    Wout = np.asarray(Wout, np.float32)

    import ml_dtypes

    tri = np.where(
        np.arange(128)[:, None] <= np.arange(128)[None, :], 0.0, NEG
    ).astype(np.float32)
    ident = np.eye(128, dtype=ml_dtypes.bfloat16)

    in_maps = []
    for c in range(CORES):
        b, g = divmod(c, 4)
        cs = 256 * g  # local col start within each of q/k/v blocks
        wq = Wqkv[:, cs : cs + 256]
        wk = Wqkv[:, D + cs : D + cs + 256]
        wv = Wqkv[:, 2 * D + cs : 2 * D + cs + 256]
        w_local = np.ascontiguousarray(np.concatenate([wq, wk, wv], axis=1))
        b_local = np.concatenate(
            [bqkv[cs : cs + 256], bqkv[D + cs : D + cs + 256], bqkv[2 * D + cs : 2 * D + cs + 256]]
        )
        bqkv_pc = np.ascontiguousarray(b_local.reshape(6, 128).T)
        wout_l = np.ascontiguousarray(Wout[cs : cs + 256, :])
        m = attention_mask[b].astype(np.float32)
        kb = np.where(m > 0, 0.0, NEG).astype(np.float32)
        kbias_pc = np.ascontiguousarray(kb.reshape(N_KCH, 128).T)
        qmask_rep = np.ascontiguousarray(np.broadcast_to(m[None, :], (128, S)))
        in_maps.append(
            {
                "xT": rnd(x[b].T),
                "wqkv": rnd(w_local),
                "bqkv_pc": bqkv_pc,
                "wout": rnd(wout_l),
                "kbias": kbias_pc,
                "qmask_rep": qmask_rep,
                "tri": tri,
                "ident": ident,
            }
        )
    return in_maps


_NC_CACHE = {}


def _get_nc(mm_dt="f32r", p_dt="bf16"):
    key = (mm_dt, p_dt)
    if key not in _NC_CACHE:
        _NC_CACHE[key] = build_nc(*key)
    return _NC_CACHE[key]


def kernel(x, attention_mask, Wqkv, bqkv, Wout, bout, _trace=False, _trace_kwargs=None):
    bout = np.asarray(bout, np.float32)
    mm_dt = os.environ.get("ATTN_MM_DT", "f32r")
    p_dt = os.environ.get("ATTN_P_DT", "bf16")
    in_maps = make_in_maps(x, attention_mask, Wqkv, bqkv, Wout, mm_dt=mm_dt)
    nc = _get_nc(mm_dt, p_dt)
    res = run_bass_kernel_spmd(
        nc,
        in_maps,
        list(range(CORES)),
        trace=_trace,
        **(_trace_kwargs or {}),
    )
    outs = [res.results[c]["out"] for c in range(CORES)]
    full = np.empty((B, S, D), np.float32)
    for b in range(B):
        full[b] = outs[4 * b] + outs[4 * b + 1] + outs[4 * b + 2] + outs[4 * b + 3] + bout
    if _trace:
        return full, res
    return full


# revision 20
# speedup vs baseline: 1.2787x; 1.0043x over previous
"""Trainium2 Bass kernel for causal self-attention (B=2, S=2048, D=1024, H=16).

Sharding: 8 cores = 2 batch groups x 4 head-groups (tensor parallel).
Core c handles batch b = c // 4 and heads [4*(c%4), 4*(c%4)+4).
Each core computes a partial out-projection [S, D]; the host sums the 4
partials of each batch group (row-parallel TP unshard) and adds bout.

Per-core pipeline (all layouts chosen so no on-device transposes of
activations are needed except small V blocks):
  1. qkvT[col, s] = Wqkv_local.T @ x.T   (x passed pre-transposed, a host
     layout choice; weights are naturally [D, cols] = lhsT layout)
  2. scoresT[k, q] = K_h^T.T @ Q_h per 128-wide k-chunk, causal blocks only.
     Key-padding mask + 1/sqrt(64) scale fold into the ACT exp (per-partition
     bias = per-k bias in this transposed layout).  P = exp(scores') in bf16.
  3. attT[65, q] = V_ext^T @ P  where V_ext = [V_h | ones]: row 64 is the
     softmax denominator.  No separate reduction needed.
  4. normalize per-q: recip = 1/(den + eps) replicated via a K=1 matmul;
     att_n = attT * recip; query-padding mask applied as one big multiply.
  5. out_partial[s, :] = att_n.T @ Wout_local  (att_n is already the lhsT
     layout needed), DMA PSUM -> DRAM directly.
"""

import os
import sys

import numpy as np

for _p in ("/opt/trn_rl_repo",):
    if _p not in sys.path and os.path.isdir(_p):
        sys.path.insert(0, _p)

import concourse.bass as bass
import concourse.mybir as mybir
from concourse import tile
from concourse.bass_utils import run_bass_kernel_spmd

B, S, D, H = 2, 2048, 1024, 16
HD = D // H  # 64
HEADS_PER_CORE = 4
CORES = 8
LOCAL_COLS = 3 * HEADS_PER_CORE * HD  # 768 (q|k|v for 4 heads)
NEG = -1.0e30
EPS = 1.0e-9  # within ACT-reciprocal valid range +-[2^-42, 2^42]

F32 = mybir.dt.float32
F32R = mybir.dt.float32r
BF16 = mybir.dt.bfloat16

AF = mybir.ActivationFunctionType

N_STILE = 4  # 512-wide s tiles
N_KCH = S // 128  # 16 k-chunks
VEXT_W = HEADS_PER_CORE * (HD + 1)  # 260


def round_f32r(a):
    """Round fp32 array to fp32r (11-bit mantissa, round-to-nearest-even)."""
    u = np.ascontiguousarray(a, np.float32).view(np.uint32)
    low = u & np.uint32(0x00000FFF)
    base = u & np.uint32(0xFFFFF000)
    lsb = (u >> np.uint32(12)) & np.uint32(1)
    up = (low > 0x800) | ((low == 0x800) & (lsb == 1))
    return (base + (up.astype(np.uint32) << np.uint32(12))).view(np.float32)



def _split_waits(nc, cap=1):
    """Walrus in this container allows few sync-waits per instruction.
    Hoist excess waits onto preceding same-engine NoOps (same sequencer,
    program order => semantics preserved).  fp32-path Matmult lowers to
    LDW+MM whose LW struct takes no waits at all -> cap 0."""
    uid = [0]
    for fn in nc.m.functions:
        for bb in fn.blocks:
            insts = bb.instructions
            out = []
            for ins in insts:
                icap = 0 if isinstance(ins, mybir.InstMatmult) else cap
                si = ins.sync_info
                waits = list(si.on_wait) if (si and si.on_wait) else []
                if len(waits) > icap:
                    extra = waits[:-icap] if icap else waits
                    keep = waits[-icap:] if icap else []
                    gcap = max(cap, 1)
                    for i in range(0, len(extra), gcap):
                        grp = extra[i : i + gcap]
                        nop = mybir.InstNoOp(
                            name=f"wsplit-{uid[0]}", ins=[], outs=[]
                        )
                        uid[0] += 1
                        nop.engine = ins.engine
                        nop.sync_info = mybir.SyncInfo(on_wait=grp, on_update=[])
                        out.append(nop)
                    si.on_wait = keep
                out.append(ins)
            if len(out) != len(insts):
                insts[:] = out
    return nc


# score-chunk table: per tj (= j//4), list of (start_col, width) chunks of
# the valid q-range [512*tj, 2048), each <= 1024 wide, 512-aligned pieces
CHUNKS = {
    0: [(0, 1024), (1024, 1024)],
    1: [(512, 512), (1024, 1024)],
    2: [(1024, 1024)],
    3: [(1536, 512)],
}


def _chunk_for(tj, col):
    for cs, cw in CHUNKS[tj]:
        if cs <= col < cs + cw:
            return cs, cw
    raise ValueError((tj, col))


def _act_recip(nc, out_ap, in_ap):
    """ACT-engine reciprocal (bass blocks ActivationFunctionType.Reciprocal
    behind an accuracy warning; ~1e-5 rel err is fine for this kernel and it
    replaces a 53us DVE InstReciprocal with one ~2us ACTIVATE)."""
    eng = nc.scalar
    inputs = [eng.lower_ap(in_ap)]
    for v in (0.0, 1.0, 0.0):  # bias, scale, alpha
        inputs.append(mybir.ImmediateValue(dtype=mybir.dt.float32, value=v))
    return eng.add_instruction(
        mybir.InstActivation(
            name=eng.bass.get_next_instruction_name(),
            func=mybir.ActivationFunctionType.Reciprocal,
            ins=inputs,
            outs=[eng.lower_ap(out_ap)],
        )
    )


def build_nc(mm_dt="f32r", p_dt="bf16", split_waits=True):
    """Build the SPMD single-core program (same program on all 8 cores)."""
    nc = bass.Bass()
    mdt = F32R if mm_dt == "f32r" else F32
    pdt = BF16 if p_dt == "bf16" else F32
    scale = float(HD) ** -0.5

    xT = nc.dram_tensor("xT", [D, S], mdt, kind="ExternalInput")
    wqkv = nc.dram_tensor("wqkv", [D, LOCAL_COLS], mdt, kind="ExternalInput")
    bqkv_pc = nc.dram_tensor("bqkv_pc", [128, 6], F32, kind="ExternalInput")
    wout = nc.dram_tensor("wout", [256, D], mdt, kind="ExternalInput")
    kbias = nc.dram_tensor("kbias", [128, N_KCH], F32, kind="ExternalInput")
    qmask_rep = nc.dram_tensor("qmask_rep", [128, S], F32, kind="ExternalInput")
    tri = nc.dram_tensor("tri", [128, 128], F32, kind="ExternalInput")
    ident = nc.dram_tensor("ident", [128, 128], pdt, kind="ExternalInput")
    out = nc.dram_tensor("out", [S, D], F32, kind="ExternalOutput")

    with tile.TileContext(nc) as tc:
        with (
            tc.tile_pool(name="consts", bufs=1) as consts,
            tc.tile_pool(name="persist", bufs=1) as persist,
        ):
            # ---- constants / persistent SBUF ----
            wout_sb = consts.tile([128, 2 * D], mdt)
            for ch in range(2):
                nc.sync.dma_start(
                    wout_sb[:, ch * D : (ch + 1) * D],
                    wout[ch * 128 : (ch + 1) * 128, :],
                )
            kbias_sb = consts.tile([128, N_KCH], F32)
            nc.sync.dma_start(kbias_sb[:], kbias[:])
            qmask_sb = consts.tile([128, S], F32)
            nc.sync.dma_start(qmask_sb[:], qmask_rep[:])
            tri_sb = consts.tile([128, 128], F32)
            nc.sync.dma_start(tri_sb[:], tri[:])

            # qkvT: 6 col-chunks x [128, S] in bf16; 0,1 = q, 2,3 = k, 4,5 = v
            qkvT = persist.tile([128, 6 * S], pdt)
            # V_ext: per k-chunk [128, 260]: 4 heads x (64 V cols + ones col)
            v_ext = persist.tile([128, N_KCH * VEXT_W], pdt)
            # att_u: attended (transposed), unnormalized then normalized in place
            att_u = persist.tile([128, 2 * S], mdt)
            # denominators: one row per head at partition h*32 (engine start-
            # partition constraint: must be 0/32/64/96)
            den4 = persist.tile([128, S], F32)
            recip4 = persist.tile([128, S], F32)

            # ==================== Phase A: QKV ====================
            with (
                tc.tile_pool(name="aconsts", bufs=1) as aconsts,
                tc.tile_pool(name="xs", bufs=3) as xs,
                tc.tile_pool(name="qkv_ps", bufs=6, space="PSUM") as qkv_ps,
                tc.tile_pool(name="tr_ps", bufs=2, space="PSUM") as tr_ps,
            ):
                wqkv_sb = aconsts.tile([128, 8 * LOCAL_COLS], mdt)
                for d in range(8):
                    nc.sync.dma_start(
                        wqkv_sb[:, d * LOCAL_COLS : (d + 1) * LOCAL_COLS],
                        wqkv[d * 128 : (d + 1) * 128, :],
                    )
                bqkv_sb = aconsts.tile([128, 6], F32)
                nc.sync.dma_start(bqkv_sb[:], bqkv_pc[:])
                ident_sb = aconsts.tile([128, 128], pdt)
                nc.sync.dma_start(ident_sb[:], ident[:])
                for t in range(N_STILE):
                    ps = [qkv_ps.tile([128, 512], F32, tag="qkvps", name=f"qkvps_{t}_{i}") for i in range(6)]
                    for d in range(8):
                        xt = xs.tile([128, 512], mdt, tag="xs", name=f"xs_{t}_{d}")
                        nc.gpsimd.dma_start(
                            xt[:], xT[d * 128 : (d + 1) * 128, t * 512 : (t + 1) * 512]
                        )
                        for cc in range(6):
                            nc.tensor.matmul(
                                ps[cc][:],
                                wqkv_sb[:, d * LOCAL_COLS + cc * 128 : d * LOCAL_COLS + (cc + 1) * 128],
                                xt[:],
                                start=(d == 0),
                                stop=(d == 7),
                            )
                    for cc in range(6):
                        nc.vector.tensor_scalar_add(
                            qkvT[:, cc * S + t * 512 : cc * S + (t + 1) * 512],
                            ps[cc][:],
                            bqkv_sb[:, cc : cc + 1],
                        )

                # V transposes: vT chunks 4,5 -> V_ext natural layout (+ones)
                for sc in range(N_KCH):
                    base = sc * VEXT_W
                    nc.any.memset(
                        v_ext[:, base : base + VEXT_W].rearrange(
                            "p (h c) -> p h c", h=HEADS_PER_CORE
                        )[:, :, HD : HD + 1],
                        1.0,
                    )
                    for hp in range(2):  # head pairs
                        tp = tr_ps.tile([128, 128], pdt, tag="trps", name=f"trps_{sc}_{hp}")
                        nc.tensor.transpose(
                            tp[:],
                            qkvT[:, (4 + hp) * S + sc * 128 : (4 + hp) * S + (sc + 1) * 128],
                            ident_sb[:],
                        )
                        nc.vector.tensor_copy(
                            v_ext[:, base + hp * 2 * (HD + 1) : base + (hp * 2 + 2) * (HD + 1)]
                            .rearrange("p (h c) -> p h c", h=2)[:, :, 0:HD],
                            tp[:].rearrange("p (h c) -> p h c", h=2),
                        )

            # ==================== Phase B: attention ====================
            with (
                tc.tile_pool(name="sc_ps", bufs=2, space="PSUM") as sc_ps,
                tc.tile_pool(name="av_ps", bufs=2, space="PSUM") as av_ps,
                tc.tile_pool(name="out_ps", bufs=2, space="PSUM") as out_ps,
                tc.tile_pool(name="pt", bufs=4) as ptp,
                tc.tile_pool(name="rr", bufs=3) as rrp,
                tc.tile_pool(name="outsb", bufs=2) as outsb,
                tc.tile_pool(name="dram", bufs=1, space="DRAM") as dramp,
            ):
                recip4_dram = dramp.tile([4, S], F32, name="recip4_dram")
                def emit_scores(h, j):
                    qrow = (h % 2) * 64
                    qch = h // 2
                    kch = 2 + h // 2
                    tj = j // 4
                    for ci, (cs, cw) in enumerate(CHUNKS[tj]):
                        sps = sc_ps.tile(
                            [128, 1024], F32, tag="scps", name=f"scps_{h}_{j}_{ci}"
                        )
                        for o in range(0, cw, 512):
                            t = (cs + o) // 512
                            nc.tensor.matmul(
                                sps[:, o : o + 512],
                                qkvT[qrow : qrow + 64, kch * S + j * 128 : kch * S + (j + 1) * 128],
                                qkvT[qrow : qrow + 64, qch * S + t * 512 : qch * S + (t + 1) * 512],
                                start=True,
                                stop=True,
                            )
                        pt = ptp.tile(
                            [128, cw], pdt, tag=f"pt{cw}",
                            bufs=(32 if cw == 1024 else 16),
                            name=f"pt_{h}_{j}_{ci}",
                        )
                        if ci == 0:
                            db = j * 128 - cs  # diag block offset in chunk
                            nc.vector.tensor_add(
                                sps[:, db : db + 128], sps[:, db : db + 128], tri_sb[:]
                            )
                            if db > 0:
                                nc.any.memset(pt[:, 0:db], 0.0)
                            nc.scalar.activation(
                                pt[:, db:cw], sps[:, db:cw], AF.Exp,
                                bias=kbias_sb[:, j : j + 1], scale=scale,
                            )
                        else:
                            nc.scalar.activation(
                                pt[:, 0:cw], sps[:, 0:cw], AF.Exp,
                                bias=kbias_sb[:, j : j + 1], scale=scale,
                            )
                        pts[(h, j, cs)] = pt

                def emit_av(h, t):
                    qrow = (h % 2) * 64
                    qch = h // 2
                    aps = av_ps.tile([65, 512], F32, tag="avps", name=f"avps_{h}_{t}")
                    jmax = 4 * t + 3
                    for j in range(jmax + 1):
                        tj = j // 4
                        cs, cw = _chunk_for(tj, t * 512)
                        nc.tensor.matmul(
                            aps[:],
                            v_ext[:, j * VEXT_W + h * (HD + 1) : j * VEXT_W + (h + 1) * (HD + 1)],
                            pts[(h, j, cs)][:, t * 512 - cs : t * 512 - cs + 512],
                            start=(j == 0),
                            stop=(j == jmax),
                        )
                    nc.vector.tensor_scalar_add(
                        den4[h * 32 : h * 32 + 1, t * 512 : (t + 1) * 512],
                        aps[64:65, :],
                        EPS,
                    )
                    nc.scalar.activation(
                        att_u[qrow : qrow + 64, qch * S + t * 512 : qch * S + (t + 1) * 512],
                        aps[0:64, :],
                        AF.Identity,
                    )

                pts = {}
                # software pipeline: head h scores interleaved with head h-1 AV
                # (AV matmuls depend only on already-exp'd P tiles, so they fill
                # the PE stalls where scores wait on exp to free PSUM slots)
                for h in range(HEADS_PER_CORE):
                    for j in range(N_KCH):
                        emit_scores(h, j)
                        if h > 0 and j % 4 == 3:
                            emit_av(h - 1, j // 4)
                for t in range(4):
                    emit_av(HEADS_PER_CORE - 1, t)

                # ---- batched reciprocals (back-to-back: one table switch),
                # with the query-padding mask folded into the recip rows ----
                for h in range(HEADS_PER_CORE):
                    _act_recip(
                        nc,
                        recip4[h * 32 : h * 32 + 1, :],
                        den4[h * 32 : h * 32 + 1, :],
                    )
                    nc.vector.tensor_mul(
                        recip4[h * 32 : h * 32 + 1, :],
                        recip4[h * 32 : h * 32 + 1, :],
                        qmask_sb[h * 32 : h * 32 + 1, :],
                    )
                nc.sync.dma_start(
                    recip4_dram[:],
                    recip4[:].rearrange("(a b) c -> a b c", b=32)[:, 0:1, :].rearrange(
                        "a b c -> (a b) c"
                    ),
                )

                # ---- normalize in place + out-projection, pipelined per q-tile ----
                for t in range(4):
                    for qch in range(2):
                        rr = rrp.tile([128, 512], F32, tag="rr", name=f"rr_{qch}_{t}")
                        for hh in range(2):
                            h = qch * 2 + hh
                            nc.sync.dma_start(
                                rr[hh * 64 : (hh + 1) * 64, :],
                                recip4_dram[h : h + 1, t * 512 : (t + 1) * 512].to_broadcast((64, 512)),
                            )
                        sl = att_u[:, qch * S + t * 512 : qch * S + (t + 1) * 512]
                        nc.vector.tensor_mul(sl, sl, rr[:])
                    for st in range(4 * t, 4 * t + 4):
                        for n in range(2):
                            ops = out_ps.tile([128, 512], F32, tag="outps", name=f"outps_{st}_{n}")
                            for ch in range(2):
                                nc.tensor.matmul(
                                    ops[:],
                                    att_u[:, ch * S + st * 128 : ch * S + (st + 1) * 128],
                                    wout_sb[:, ch * D + n * 512 : ch * D + (n + 1) * 512],
                                    start=(ch == 0),
                                    stop=(ch == 1),
                                )
                            osb = outsb.tile([128, 512], F32, tag="outsb", name=f"outsb_{st}_{n}")
                            nc.vector.tensor_copy(osb[:], ops[:])
                            nc.sync.dma_start(
                                out[st * 128 : (st + 1) * 128, n * 512 : (n + 1) * 512],
                                osb[:],
                            )

    return _split_waits(nc) if split_waits else nc


def make_in_maps(x, attention_mask, Wqkv, bqkv, Wout, mm_dt="f32r"):
    """Shard full inputs into the 8 per-core input dicts."""
    rnd = round_f32r if mm_dt == "f32r" else (lambda a: np.ascontiguousarray(a, np.float32))
    x = np.asarray(x, np.float32)
    attention_mask = np.asarray(attention_mask)
    Wqkv = np.asarray(Wqkv, np.float32)
    bqkv = np.asarray(bqkv, np.float32)
    Wout = np.asarray(Wout, np.float32)

    import ml_dtypes

    tri = np.where(
        np.arange(128)[:, None] <= np.arange(128)[None, :], 0.0, NEG
    ).astype(np.float32)
    ident = np.eye(128, dtype=ml_dtypes.bfloat16)

    in_maps = []
    for c in range(CORES):
        b, g = divmod(c, 4)
        cs = 256 * g  # local col start within each of q/k/v blocks
        wq = Wqkv[:, cs : cs + 256]
        wk = Wqkv[:, D + cs : D + cs + 256]
        wv = Wqkv[:, 2 * D + cs : 2 * D + cs + 256]
        w_local = np.ascontiguousarray(np.concatenate([wq, wk, wv], axis=1))
        b_local = np.concatenate(
            [bqkv[cs : cs + 256], bqkv[D + cs : D + cs + 256], bqkv[2 * D + cs : 2 * D + cs + 256]]
        )
        bqkv_pc = np.ascontiguousarray(b_local.reshape(6, 128).T)
        wout_l = np.ascontiguousarray(Wout[cs : cs + 256, :])
        m = attention_mask[b].astype(np.float32)
        kb = np.where(m > 0, 0.0, NEG).astype(np.float32)
        kbias_pc = np.ascontiguousarray(kb.reshape(N_KCH, 128).T)
        qmask_rep = np.ascontiguousarray(np.broadcast_to(m[None, :], (128, S)))
        in_maps.append(
            {
                "xT": rnd(x[b].T),
                "wqkv": rnd(w_local),
                "bqkv_pc": bqkv_pc,
                "wout": rnd(wout_l),
                "kbias": kbias_pc,
                "qmask_rep": qmask_rep,
                "tri": tri,
                "ident": ident,
            }
        )
    return in_maps


_NC_CACHE = {}


def _get_nc(mm_dt="f32r", p_dt="bf16"):
    key = (mm_dt, p_dt)
    if key not in _NC_CACHE:
        _NC_CACHE[key] = build_nc(*key)
    return _NC_CACHE[key]


def kernel(x, attention_mask, Wqkv, bqkv, Wout, bout, _trace=False, _trace_kwargs=None):
    bout = np.asarray(bout, np.float32)
    mm_dt = os.environ.get("ATTN_MM_DT", "f32r")
    p_dt = os.environ.get("ATTN_P_DT", "bf16")
    in_maps = make_in_maps(x, attention_mask, Wqkv, bqkv, Wout, mm_dt=mm_dt)
    nc = _get_nc(mm_dt, p_dt)
    res = run_bass_kernel_spmd(
        nc,
        in_maps,
        list(range(CORES)),
        trace=_trace,
        **(_trace_kwargs or {}),
    )
    outs = [res.results[c]["out"] for c in range(CORES)]
    full = np.empty((B, S, D), np.float32)
    for b in range(B):
        full[b] = outs[4 * b] + outs[4 * b + 1] + outs[4 * b + 2] + outs[4 * b + 3] + bout
    if _trace:
        return full, res
    return full


# revision 21
# speedup vs baseline: 1.3752x; 1.0755x over previous
"""Trainium2 Bass kernel for causal self-attention (B=2, S=2048, D=1024, H=16).

Sharding: 8 cores = 2 batch groups x 4 head-groups (tensor parallel).
Core c handles batch b = c // 4 and heads [4*(c%4), 4*(c%4)+4).
Each core computes a partial out-projection [S, D]; the host sums the 4
partials of each batch group (row-parallel TP unshard) and adds bout.

Per-core pipeline (all layouts chosen so no on-device transposes of
activations are needed except small V blocks):
  1. qkvT[col, s] = Wqkv_local.T @ x.T   (x passed pre-transposed, a host
     layout choice; weights are naturally [D, cols] = lhsT layout)
  2. scoresT[k, q] = K_h^T.T @ Q_h per 128-wide k-chunk, causal blocks only.
     Key-padding mask + 1/sqrt(64) scale fold into the ACT exp (per-partition
     bias = per-k bias in this transposed layout).  P = exp(scores') in bf16.
  3. attT[65, q] = V_ext^T @ P  where V_ext = [V_h | ones]: row 64 is the
     softmax denominator.  No separate reduction needed.
  4. normalize per-q: recip = 1/(den + eps) replicated via a K=1 matmul;
     att_n = attT * recip; query-padding mask applied as one big multiply.
  5. out_partial[s, :] = att_n.T @ Wout_local  (att_n is already the lhsT
     layout needed), DMA PSUM -> DRAM directly.
"""

import os
import sys

import numpy as np

for _p in ("/opt/trn_rl_repo",):
    if _p not in sys.path and os.path.isdir(_p):
        sys.path.insert(0, _p)

import concourse.bass as bass
import concourse.mybir as mybir
from concourse import tile
from concourse.bass_utils import run_bass_kernel_spmd

B, S, D, H = 2, 2048, 1024, 16
HD = D // H  # 64
HEADS_PER_CORE = 4
CORES = 8
LOCAL_COLS = 3 * HEADS_PER_CORE * HD  # 768 (q|k|v for 4 heads)
NEG = -1.0e30
EPS = 1.0e-9  # within ACT-reciprocal valid range +-[2^-42, 2^42]

F32 = mybir.dt.float32
F32R = mybir.dt.float32r
BF16 = mybir.dt.bfloat16

AF = mybir.ActivationFunctionType

N_STILE = 4  # 512-wide s tiles
N_KCH = S // 128  # 16 k-chunks
VEXT_W = HEADS_PER_CORE * (HD + 1)  # 260


def round_f32r(a):
    """Round fp32 array to fp32r (11-bit mantissa, round-to-nearest-even)."""
    u = np.ascontiguousarray(a, np.float32).view(np.uint32)
    low = u & np.uint32(0x00000FFF)
    base = u & np.uint32(0xFFFFF000)
    lsb = (u >> np.uint32(12)) & np.uint32(1)
    up = (low > 0x800) | ((low == 0x800) & (lsb == 1))
    return (base + (up.astype(np.uint32) << np.uint32(12))).view(np.float32)



def _split_waits(nc, cap=1):
    """Walrus in this container allows few sync-waits per instruction.
    Hoist excess waits onto preceding same-engine NoOps (same sequencer,
    program order => semantics preserved).  fp32-path Matmult lowers to
    LDW+MM whose LW struct takes no waits at all -> cap 0."""
    uid = [0]
    for fn in nc.m.functions:
        for bb in fn.blocks:
            insts = bb.instructions
            out = []
            for ins in insts:
                icap = 0 if isinstance(ins, mybir.InstMatmult) else cap
                si = ins.sync_info
                waits = list(si.on_wait) if (si and si.on_wait) else []
                if len(waits) > icap:
                    extra = waits[:-icap] if icap else waits
                    keep = waits[-icap:] if icap else []
                    gcap = max(cap, 1)
                    for i in range(0, len(extra), gcap):
                        grp = extra[i : i + gcap]
                        nop = mybir.InstNoOp(
                            name=f"wsplit-{uid[0]}", ins=[], outs=[]
                        )
                        uid[0] += 1
                        nop.engine = ins.engine
                        nop.sync_info = mybir.SyncInfo(on_wait=grp, on_update=[])
                        out.append(nop)
                    si.on_wait = keep
                out.append(ins)
            if len(out) != len(insts):
                insts[:] = out
    return nc


# score-chunk table: per tj (= j//4), list of (start_col, width) chunks of
# the valid q-range [512*tj, 2048), each <= 1024 wide, 512-aligned pieces
CHUNKS = {
    0: [(0, 1024), (1024, 1024)],
    1: [(512, 512), (1024, 1024)],
    2: [(1024, 1024)],
    3: [(1536, 512)],
}


def _chunk_for(tj, col):
    for cs, cw in CHUNKS[tj]:
        if cs <= col < cs + cw:
            return cs, cw
    raise ValueError((tj, col))


def _act_recip(nc, out_ap, in_ap):
    """ACT-engine reciprocal (bass blocks ActivationFunctionType.Reciprocal
    behind an accuracy warning; ~1e-5 rel err is fine for this kernel and it
    replaces a 53us DVE InstReciprocal with one ~2us ACTIVATE)."""
    eng = nc.scalar
    inputs = [eng.lower_ap(in_ap)]
    for v in (0.0, 1.0, 0.0):  # bias, scale, alpha
        inputs.append(mybir.ImmediateValue(dtype=mybir.dt.float32, value=v))
    return eng.add_instruction(
        mybir.InstActivation(
            name=eng.bass.get_next_instruction_name(),
            func=mybir.ActivationFunctionType.Reciprocal,
            ins=inputs,
            outs=[eng.lower_ap(out_ap)],
        )
    )


def build_nc(mm_dt="f32r", p_dt="bf16", split_waits=True):
    """Build the SPMD single-core program (same program on all 8 cores)."""
    nc = bass.Bass()
    mdt = F32R if mm_dt == "f32r" else F32
    pdt = BF16 if p_dt == "bf16" else F32
    scale = float(HD) ** -0.5

    xT = nc.dram_tensor("xT", [D, S], mdt, kind="ExternalInput")
    wqkv = nc.dram_tensor("wqkv", [D, LOCAL_COLS], mdt, kind="ExternalInput")
    bqkv_pc = nc.dram_tensor("bqkv_pc", [128, 6], F32, kind="ExternalInput")
    wout = nc.dram_tensor("wout", [256, D], mdt, kind="ExternalInput")
    kbias = nc.dram_tensor("kbias", [128, N_KCH], F32, kind="ExternalInput")
    qmask_rep = nc.dram_tensor("qmask_rep", [128, S], F32, kind="ExternalInput")
    tri = nc.dram_tensor("tri", [128, 128], F32, kind="ExternalInput")
    ident = nc.dram_tensor("ident", [128, 128], pdt, kind="ExternalInput")
    out = nc.dram_tensor("out", [S, D], F32, kind="ExternalOutput")

    with tile.TileContext(nc) as tc:
        with (
            tc.tile_pool(name="consts", bufs=1) as consts,
            tc.tile_pool(name="persist", bufs=1) as persist,
        ):
            # ---- constants / persistent SBUF ----
            wout_sb = consts.tile([128, 2 * D], mdt)
            for ch in range(2):
                nc.sync.dma_start(
                    wout_sb[:, ch * D : (ch + 1) * D],
                    wout[ch * 128 : (ch + 1) * 128, :],
                )
            kbias_sb = consts.tile([128, N_KCH], F32)
            nc.sync.dma_start(kbias_sb[:], kbias[:])
            qmask_sb = consts.tile([128, S], F32)
            nc.sync.dma_start(qmask_sb[:], qmask_rep[:])
            tri_sb = consts.tile([128, 128], F32)
            nc.sync.dma_start(tri_sb[:], tri[:])

            # qkvT: 6 col-chunks x [128, S] in bf16; 0,1 = q, 2,3 = k, 4,5 = v
            qkvT = persist.tile([128, 6 * S], pdt)
            # V_ext: per k-chunk [128, 260]: 4 heads x (64 V cols + ones col)
            v_ext = persist.tile([128, N_KCH * VEXT_W], pdt)
            # att_u: attended (transposed), unnormalized then normalized in place
            att_u = persist.tile([128, 2 * S], mdt)
            # denominators: one row per head at partition h*32 (engine start-
            # partition constraint: must be 0/32/64/96)
            den4 = persist.tile([128, S], F32)
            recip4 = persist.tile([128, S], F32)

            # ==================== Phase A: QKV ====================
            with (
                tc.tile_pool(name="aconsts", bufs=1) as aconsts,
                tc.tile_pool(name="xs", bufs=3) as xs,
                tc.tile_pool(name="qkv_ps", bufs=6, space="PSUM") as qkv_ps,
                tc.tile_pool(name="tr_ps", bufs=2, space="PSUM") as tr_ps,
            ):
                wqkv_sb = aconsts.tile([128, 8 * LOCAL_COLS], mdt)
                for d in range(8):
                    nc.sync.dma_start(
                        wqkv_sb[:, d * LOCAL_COLS : (d + 1) * LOCAL_COLS],
                        wqkv[d * 128 : (d + 1) * 128, :],
                    )
                bqkv_sb = aconsts.tile([128, 6], F32)
                nc.sync.dma_start(bqkv_sb[:], bqkv_pc[:])
                ident_sb = aconsts.tile([128, 128], pdt)
                nc.sync.dma_start(ident_sb[:], ident[:])
                for t in range(N_STILE):
                    ps = [qkv_ps.tile([128, 512], F32, tag="qkvps", name=f"qkvps_{t}_{i}") for i in range(6)]
                    for d in range(8):
                        xt = xs.tile([128, 512], mdt, tag="xs", name=f"xs_{t}_{d}")
                        nc.gpsimd.dma_start(
                            xt[:], xT[d * 128 : (d + 1) * 128, t * 512 : (t + 1) * 512]
                        )
                        for cc in range(6):
                            nc.tensor.matmul(
                                ps[cc][:],
                                wqkv_sb[:, d * LOCAL_COLS + cc * 128 : d * LOCAL_COLS + (cc + 1) * 128],
                                xt[:],
                                start=(d == 0),
                                stop=(d == 7),
                            )
                    for cc in range(6):
                        nc.vector.tensor_scalar_add(
                            qkvT[:, cc * S + t * 512 : cc * S + (t + 1) * 512],
                            ps[cc][:],
                            bqkv_sb[:, cc : cc + 1],
                        )

                # V transposes: vT chunks 4,5 -> V_ext natural layout (+ones)
                for sc in range(N_KCH):
                    base = sc * VEXT_W
                    nc.any.memset(
                        v_ext[:, base : base + VEXT_W].rearrange(
                            "p (h c) -> p h c", h=HEADS_PER_CORE
                        )[:, :, HD : HD + 1],
                        1.0,
                    )
                    for hp in range(2):  # head pairs
                        tp = tr_ps.tile([128, 128], pdt, tag="trps", name=f"trps_{sc}_{hp}")
                        nc.tensor.transpose(
                            tp[:],
                            qkvT[:, (4 + hp) * S + sc * 128 : (4 + hp) * S + (sc + 1) * 128],
                            ident_sb[:],
                        )
                        nc.vector.tensor_copy(
                            v_ext[:, base + hp * 2 * (HD + 1) : base + (hp * 2 + 2) * (HD + 1)]
                            .rearrange("p (h c) -> p h c", h=2)[:, :, 0:HD],
                            tp[:].rearrange("p (h c) -> p h c", h=2),
                        )

            # ==================== Phase B: attention ====================
            with (
                tc.tile_pool(name="sc_ps", bufs=3, space="PSUM") as sc_ps,
                tc.tile_pool(name="av_ps", bufs=2, space="PSUM") as av_ps,
                tc.tile_pool(name="pt", bufs=4) as ptp,
                tc.tile_pool(name="rr", bufs=3) as rrp,
                tc.tile_pool(name="outsb", bufs=2) as outsb,
                tc.tile_pool(name="dram", bufs=1, space="DRAM") as dramp,
            ):
                recip4_dram = dramp.tile([4, S], F32, name="recip4_dram")
                def emit_scores_pair(p, j):
                    qch = p
                    kch = 2 + p
                    tj = j // 4
                    for ci, (cs, cw) in enumerate(CHUNKS[tj]):
                        tiles = []
                        for hh in range(2):
                            h = 2 * p + hh
                            qrow = hh * 64
                            sps = sc_ps.tile(
                                [128, 1024], F32, tag="scps", name=f"scps_{h}_{j}_{ci}"
                            )
                            tiles.append(sps)
                        # alternate heads per 512-slice: adjacent matmuls use
                        # disjoint row groups -> concurrent execution
                        for o in range(0, cw, 512):
                            t = (cs + o) // 512
                            for hh in range(2):
                                qrow = hh * 64
                                nc.tensor.matmul(
                                    tiles[hh][:, o : o + 512],
                                    qkvT[qrow : qrow + 64, kch * S + j * 128 : kch * S + (j + 1) * 128],
                                    qkvT[qrow : qrow + 64, qch * S + t * 512 : qch * S + (t + 1) * 512],
                                    start=True,
                                    stop=True,
                                )
                        for hh in range(2):
                            h = 2 * p + hh
                            sps = tiles[hh]
                            pt = ptp.tile(
                                [128, cw], pdt, tag=f"pt{cw}",
                                bufs=(32 if cw == 1024 else 16),
                                name=f"pt_{h}_{j}_{ci}",
                            )
                            if ci == 0:
                                db = j * 128 - cs  # diag block offset in chunk
                                nc.vector.tensor_add(
                                    sps[:, db : db + 128], sps[:, db : db + 128], tri_sb[:]
                                )
                                if db > 0:
                                    nc.any.memset(pt[:, 0:db], 0.0)
                                nc.scalar.activation(
                                    pt[:, db:cw], sps[:, db:cw], AF.Exp,
                                    bias=kbias_sb[:, j : j + 1], scale=scale,
                                )
                            else:
                                nc.scalar.activation(
                                    pt[:, 0:cw], sps[:, 0:cw], AF.Exp,
                                    bias=kbias_sb[:, j : j + 1], scale=scale,
                                )
                            pts[(h, j, cs)] = pt

                def emit_av(h, t):
                    qrow = (h % 2) * 64
                    qch = h // 2
                    aps = av_ps.tile([65, 512], F32, tag="avps", padded_shape=[128, 512], name=f"avps_{h}_{t}")
                    jmax = 4 * t + 3
                    for j in range(jmax + 1):
                        tj = j // 4
                        cs, cw = _chunk_for(tj, t * 512)
                        nc.tensor.matmul(
                            aps[:],
                            v_ext[:, j * VEXT_W + h * (HD + 1) : j * VEXT_W + (h + 1) * (HD + 1)],
                            pts[(h, j, cs)][:, t * 512 - cs : t * 512 - cs + 512],
                            start=(j == 0),
                            stop=(j == jmax),
                        )
                    nc.vector.tensor_scalar_add(
                        den4[h * 32 : h * 32 + 1, t * 512 : (t + 1) * 512],
                        aps[64:65, :],
                        EPS,
                    )
                    nc.scalar.activation(
                        att_u[qrow : qrow + 64, qch * S + t * 512 : qch * S + (t + 1) * 512],
                        aps[0:64, :],
                        AF.Identity,
                    )

                pts = {}
                # head-PAIR emission: heads 2p / 2p+1 sit at partitions 0-63 /
                # 64-127 (disjoint PE row groups), so their adjacent score
                # matmuls execute concurrently in the array.  AV for q-tile t
                # drains right after j=4t+3 inside the same pair phase, filling
                # PE stalls where scores wait on ACT exp to free PSUM slots.
                for p in range(2):
                    for j in range(N_KCH):
                        emit_scores_pair(p, j)
                        if j % 4 == 3:
                            emit_av(2 * p, j // 4)
                            emit_av(2 * p + 1, j // 4)

                # ---- batched reciprocals (back-to-back: one table switch),
                # with the query-padding mask folded into the recip rows ----
                for h in range(HEADS_PER_CORE):
                    _act_recip(
                        nc,
                        recip4[h * 32 : h * 32 + 1, :],
                        den4[h * 32 : h * 32 + 1, :],
                    )
                    nc.vector.tensor_mul(
                        recip4[h * 32 : h * 32 + 1, :],
                        recip4[h * 32 : h * 32 + 1, :],
                        qmask_sb[h * 32 : h * 32 + 1, :],
                    )
                nc.sync.dma_start(
                    recip4_dram[:],
                    recip4[:].rearrange("(a b) c -> a b c", b=32)[:, 0:1, :].rearrange(
                        "a b c -> (a b) c"
                    ),
                )

                # ---- normalize in place + out-projection, pipelined per q-tile ----
                for t in range(4):
                    for qch in range(2):
                        rr = rrp.tile([128, 512], F32, tag="rr", name=f"rr_{qch}_{t}")
                        for hh in range(2):
                            h = qch * 2 + hh
                            nc.sync.dma_start(
                                rr[hh * 64 : (hh + 1) * 64, :],
                                recip4_dram[h : h + 1, t * 512 : (t + 1) * 512].to_broadcast((64, 512)),
                            )
                        sl = att_u[:, qch * S + t * 512 : qch * S + (t + 1) * 512]
                        nc.vector.tensor_mul(sl, sl, rr[:])
                    for st in range(4 * t, 4 * t + 4):
                        for n in range(2):
                            ops = av_ps.tile([128, 512], F32, tag="avps", name=f"outps_{st}_{n}")
                            for ch in range(2):
                                nc.tensor.matmul(
                                    ops[:],
                                    att_u[:, ch * S + st * 128 : ch * S + (st + 1) * 128],
                                    wout_sb[:, ch * D + n * 512 : ch * D + (n + 1) * 512],
                                    start=(ch == 0),
                                    stop=(ch == 1),
                                )
                            osb = outsb.tile([128, 512], F32, tag="outsb", name=f"outsb_{st}_{n}")
                            nc.vector.tensor_copy(osb[:], ops[:])
                            nc.sync.dma_start(
                                out[st * 128 : (st + 1) * 128, n * 512 : (n + 1) * 512],
                                osb[:],
                            )

    return _split_waits(nc) if split_waits else nc


def make_in_maps(x, attention_mask, Wqkv, bqkv, Wout, mm_dt="f32r"):
    """Shard full inputs into the 8 per-core input dicts."""
    rnd = round_f32r if mm_dt == "f32r" else (lambda a: np.ascontiguousarray(a, np.float32))
    x = np.asarray(x, np.float32)
    attention_mask = np.asarray(attention_mask)
    Wqkv = np.asarray(Wqkv, np.float32)
    bqkv = np.asarray(bqkv, np.float32)
    Wout = np.asarray(Wout, np.float32)

    import ml_dtypes

    tri = np.where(
        np.arange(128)[:, None] <= np.arange(128)[None, :], 0.0, NEG
    ).astype(np.float32)
    ident = np.eye(128, dtype=ml_dtypes.bfloat16)

    in_maps = []
    for c in range(CORES):
        b, g = divmod(c, 4)
        cs = 256 * g  # local col start within each of q/k/v blocks
        wq = Wqkv[:, cs : cs + 256]
        wk = Wqkv[:, D + cs : D + cs + 256]
        wv = Wqkv[:, 2 * D + cs : 2 * D + cs + 256]
        w_local = np.ascontiguousarray(np.concatenate([wq, wk, wv], axis=1))
        b_local = np.concatenate(
            [bqkv[cs : cs + 256], bqkv[D + cs : D + cs + 256], bqkv[2 * D + cs : 2 * D + cs + 256]]
        )
        bqkv_pc = np.ascontiguousarray(b_local.reshape(6, 128).T)
        wout_l = np.ascontiguousarray(Wout[cs : cs + 256, :])
        m = attention_mask[b].astype(np.float32)
        kb = np.where(m > 0, 0.0, NEG).astype(np.float32)
        kbias_pc = np.ascontiguousarray(kb.reshape(N_KCH, 128).T)
        qmask_rep = np.ascontiguousarray(np.broadcast_to(m[None, :], (128, S)))
        in_maps.append(
            {
                "xT": rnd(x[b].T),
                "wqkv": rnd(w_local),
                "bqkv_pc": bqkv_pc,
                "wout": rnd(wout_l),
                "kbias": kbias_pc,
                "qmask_rep": qmask_rep,
                "tri": tri,
                "ident": ident,
            }
        )
    return in_maps


_NC_CACHE = {}


def _get_nc(mm_dt="f32r", p_dt="bf16"):
    key = (mm_dt, p_dt)
    if key not in _NC_CACHE:
        _NC_CACHE[key] = build_nc(*key)
    return _NC_CACHE[key]


def kernel(x, attention_mask, Wqkv, bqkv, Wout, bout, _trace=False, _trace_kwargs=None):
    bout = np.asarray(bout, np.float32)
    mm_dt = os.environ.get("ATTN_MM_DT", "f32r")
    p_dt = os.environ.get("ATTN_P_DT", "bf16")
    in_maps = make_in_maps(x, attention_mask, Wqkv, bqkv, Wout, mm_dt=mm_dt)
    nc = _get_nc(mm_dt, p_dt)
    res = run_bass_kernel_spmd(
        nc,
        in_maps,
        list(range(CORES)),
        trace=_trace,
        **(_trace_kwargs or {}),
    )
    outs = [res.results[c]["out"] for c in range(CORES)]
    full = np.empty((B, S, D), np.float32)
    for b in range(B):
        full[b] = outs[4 * b] + outs[4 * b + 1] + outs[4 * b + 2] + outs[4 * b + 3] + bout
    if _trace:
        return full, res
    return full
